# revision 21
# baseline (speedup 1.0000x reference)
"""Trainium2 Bass kernel for nn_Brain (7-conv CNN backbone + BN(train) +
16 per-node MLP colons with one message-passing round), data-parallel over
batch across 8 NeuronCores.

Self-contained: hardcodes all shapes/sharding. Returns (squared_sum_preds,
preds2) like the reference.
"""
import numpy as np
import ml_dtypes

import concourse.bass as bass
import concourse.mybir as mybir
import concourse.tile as tile
from concourse import bass_utils
from concourse.masks import make_identity
from concourse.vector_clock import ScopedClock

bf16 = ml_dtypes.bfloat16
F32 = mybir.dt.float32
BF = mybir.dt.bfloat16
AF = mybir.ActivationFunctionType
ALU = mybir.AluOpType
AX = mybir.AxisListType

N_CORES = 8
B = 128
BL = 16            # batch per core
NEG = -1.0e30      # -inf-ish padding for raw max-pools
EPS = 1e-5
OFF9 = [(dh, dw) for dh in range(3) for dw in range(3)]
RG = [list(range(N_CORES))]

_LAST_RESULT = None


# ---------------------------------------------------------------------------
# compat TileContext: this container's walrus accepts at most ONE sync wait
# per instruction; split extra waits onto NOPs on the same engine.
_REAL_ENGINES = {
    mybir.EngineType.PE,
    mybir.EngineType.DVE,
    mybir.EngineType.Activation,
    mybir.EngineType.Pool,
    mybir.EngineType.SP,
}


class CompatTileContext(tile.TileContext):
    MAX_SYNC = 1

    def _commit_instruction(self, inst, lazy_reg_writes=True):
        si = getattr(inst, "sync_info", None)
        if (
            si is not None
            and si.on_wait
            and len(si.on_wait) > 1
            and inst.engine in _REAL_ENGINES
        ):
            waits = list(si.on_wait)
            si.on_wait = waits[-1:]
            for k, w in enumerate(waits[:-1]):
                nop = mybir.InstNoOp(
                    name=f"{inst.name}-xw{k}",
                    sync_info=mybir.SyncInfo(on_wait=[w], on_update=[]),
                    bass_nofuse=True,
                    engine=inst.engine,
                )
                super()._commit_instruction(nop, lazy_reg_writes=False)
        return super()._commit_instruction(inst, lazy_reg_writes)

    def _drain_and_barrier(self, tick_clock, wait_clock):
        nop0 = self.nc.sync.nop(nofuse=True)
        wait_clock.add_sem_waits(nop0.ins, ScopedClock({None: tick_clock.global_clock}))
        si = nop0.ins.sync_info
        waits = list(si.on_wait) if si is not None and si.on_wait else []
        if len(waits) > self.MAX_SYNC:
            si.on_wait = waits[: self.MAX_SYNC]
            rest = waits[self.MAX_SYNC:]
            while rest:
                nop = self.nc.sync.nop(nofuse=True)
                nsi = nop.ins.sync_info
                chunk, rest = rest[: self.MAX_SYNC], rest[self.MAX_SYNC:]
                if nsi is None:
                    nop.ins.sync_info = mybir.SyncInfo(on_wait=chunk, on_update=[])
                else:
                    nsi.on_wait = chunk
        self.nc.sync.drain()

        self.nc.all_engine_barrier()
        assert self.sems is not None
        popped = self.nc._tile_sem_poison_stack.pop()
        assert popped is self._sem_poison
        self.nc.clear_and_free_semaphores(list(self.sems.allocated().values()))
        self.nc.all_engine_barrier()


# ---------------------------------------------------------------------------
# host-side weight preparation

def _neighbors(i, w, h):
    size = w * h
    out = []
    if i - w >= 0:
        out.append(i - w)
    if i % w != 0:
        out.append(i - 1)
    if (i + 1) % w != 0:
        out.append(i + 1)
    if i + w < size:
        out.append(i + w)
    if i - w - 1 >= 0 and i % w != 0:
        out.append(i - w - 1)
    if i - w + 1 >= 0 and (i + 1) % w != 0:
        out.append(i - w + 1)
    if i + w - 1 < size and i % w != 0:
        out.append(i + w - 1)
    if i + w + 1 < size and (i + 1) % w != 0:
        out.append(i + w + 1)
    return out


def _prep(inputs):
    f = np.float32
    g = {}

    cw1 = np.asarray(inputs["cw1"], f)
    g["w1t"] = np.ascontiguousarray(cw1.transpose(2, 3, 1, 0).reshape(27, 64)).astype(bf16)

    x = np.asarray(inputs["x"], f).astype(bf16)        # [128, 3, 64, 64]
    xt = x.transpose(1, 0, 2, 3)                        # [3, 128, 64, 64]
    xi = np.zeros((27, 128, 64, 64), bf16)
    for k, (dh, dw) in enumerate([(a, b) for a in range(3) for b in range(3)]):
        hs, he = max(0, 1 - dh), 64 + min(0, 1 - dh)
        ws, we = max(0, 1 - dw), 64 + min(0, 1 - dw)
        xi[3 * k:3 * k + 3, :, hs:he, ws:we] = \
            xt[:, :, hs + dh - 1:he + dh - 1, ws + dw - 1:we + dw - 1]
    g["_xi_full"] = xi

    w2 = np.asarray(inputs["cw2"], f).transpose(2, 3, 1, 0).reshape(9, 64, 128)
    g["w2t"] = np.ascontiguousarray(
        np.concatenate([w2, w2], axis=1).transpose(1, 0, 2)).astype(bf16)  # [128,9,128]

    def conv_t(cw, kt, ct):
        a = np.asarray(cw, f).transpose(2, 3, 1, 0).reshape(9, kt, 128, ct, 128)
        return np.ascontiguousarray(a.transpose(3, 2, 1, 0, 4)).astype(bf16)  # [ct,p,kt,k,q]

    g["w3t"] = conv_t(inputs["cw3"], 1, 2)
    g["w4t"] = conv_t(inputs["cw4"], 2, 2)
    g["w5t"] = conv_t(inputs["cw5"], 2, 4)
    g["w6t"] = conv_t(inputs["cw6"], 4, 4)
    g["w7t"] = conv_t(inputs["cw7"], 4, 4)

    cbs = np.zeros((128, 16), f)
    cbs[:, 0:2] = np.asarray(inputs["cb3"], f).reshape(2, 128).T
    cbs[:, 2:4] = np.asarray(inputs["cb4"], f).reshape(2, 128).T
    cbs[:, 4:8] = np.asarray(inputs["cb5"], f).reshape(4, 128).T
    cbs[:, 8:12] = np.asarray(inputs["cb6"], f).reshape(4, 128).T
    cbs[:, 12:16] = np.asarray(inputs["cb7"], f).reshape(4, 128).T
    g["cbs"] = cbs

    bng = np.zeros((128, 4), f)
    bng[:, 0] = np.tile(np.asarray(inputs["bn1_g"], f), 2)
    bng[:, 1] = np.tile(np.asarray(inputs["bn1_b"], f), 2)
    bng[:, 2] = np.asarray(inputs["bn2_g"], f)
    bng[:, 3] = np.asarray(inputs["bn2_b"], f)
    g["bng"] = bng

    W1 = np.asarray(inputs["W1"], f)        # [16, 592, 600]
    b1 = np.asarray(inputs["b1"], f)        # [16, 600]
    W2 = np.asarray(inputs["W2"], f)        # [16, 600, 10]
    b2 = np.asarray(inputs["b2"], f)        # [16, 10]

    w1f = np.zeros((16, 512, 640), f)
    w1f[:, :, :600] = W1[:, 80:, :]
    g["w1f"] = np.ascontiguousarray(
        w1f.reshape(16, 4, 128, 5, 128).transpose(2, 0, 1, 3, 4)).astype(bf16)

    idxs = np.zeros((16, 8), np.int64)
    mask = np.zeros((16, 8), f)
    for i in range(16):
        nb = _neighbors(i, 4, 4)
        idxs[i, :len(nb)] = nb
        mask[i, :len(nb)] = 1.0
    wt = np.zeros((16, 256, 640), f)
    for n in range(16):
        G = np.zeros((80, 160), f)
        for j in range(8):
            if mask[n, j] > 0:
                for c in range(10):
                    G[10 * j + c, 10 * idxs[n, j] + c] = 1.0
        wt[n, :160, :600] = G.T @ W1[n, :80, :]
    g["wt"] = np.ascontiguousarray(
        wt.reshape(16, 2, 128, 5, 128).transpose(2, 0, 1, 3, 4)).astype(bf16)
    g["b2r"] = np.ascontiguousarray(b2.reshape(1, 16, 10)).astype(bf16)

    b1p = np.zeros((16, 640), f)
    b1p[:, :600] = b1
    g["b1r"] = np.ascontiguousarray(b1p.reshape(1, 16, 5, 128)).astype(bf16)

    w2p = np.zeros((16, 640, 10), f)
    w2p[:, :600] = W2
    g["w2c"] = np.ascontiguousarray(
        w2p.reshape(16, 5, 128, 10).transpose(2, 0, 1, 3)).astype(bf16)
    return g


# ---------------------------------------------------------------------------
# device program

def _build():
    nc = bass.Bass()
    d = {}
    d["x_d"] = nc.dram_tensor("xi", [27, BL, 64, 64], BF, kind="ExternalInput")
    d["w1t_d"] = nc.dram_tensor("w1t", [27, 64], BF, kind="ExternalInput")
    d["w2t_d"] = nc.dram_tensor("w2t", [128, 9, 128], BF, kind="ExternalInput")
    d["w3t_d"] = nc.dram_tensor("w3t", [2, 128, 1, 9, 128], BF, kind="ExternalInput")
    d["w4t_d"] = nc.dram_tensor("w4t", [2, 128, 2, 9, 128], BF, kind="ExternalInput")
    d["w5t_d"] = nc.dram_tensor("w5t", [4, 128, 2, 9, 128], BF, kind="ExternalInput")
    d["w6t_d"] = nc.dram_tensor("w6t", [4, 128, 4, 9, 128], BF, kind="ExternalInput")
    d["w7t_d"] = nc.dram_tensor("w7t", [4, 128, 4, 9, 128], BF, kind="ExternalInput")
    d["cbs_d"] = nc.dram_tensor("cbs", [128, 16], F32, kind="ExternalInput")
    d["bng_d"] = nc.dram_tensor("bng", [128, 4], F32, kind="ExternalInput")
    d["w1f_d"] = nc.dram_tensor("w1f", [128, 16, 4, 5, 128], BF, kind="ExternalInput")
    d["wt_d"] = nc.dram_tensor("wt", [128, 16, 2, 5, 128], BF, kind="ExternalInput")
    d["b1r_d"] = nc.dram_tensor("b1r", [1, 16, 5, 128], BF, kind="ExternalInput")
    d["w2c_d"] = nc.dram_tensor("w2c", [128, 16, 5, 10], BF, kind="ExternalInput")
    d["b2r_d"] = nc.dram_tensor("b2r", [1, 16, 10], BF, kind="ExternalInput")
    d["out_d"] = nc.dram_tensor("preds2", [BL, 16, 10], F32, kind="ExternalOutput")

    d["cc1_in"] = nc.dram_tensor("cc1_in", [128, 2], F32)
    d["cc1_out"] = nc.dram_tensor("cc1_out", [128, 2], F32, addr_space="Shared")
    d["cc2_in"] = nc.dram_tensor("cc2_in", [128, 2], F32)
    d["cc2_out"] = nc.dram_tensor("cc2_out", [128, 2], F32, addr_space="Shared")

    with CompatTileContext(nc, pool_alloc_mode="queue") as tc:
        _trace(nc, tc, d)
    return nc


def _bn_reduce_apply(nc, sp, stats6, bng_s, cc_in, cc_out, n_shards, fold_halves,
                     gcol, bcol, tg):
    """bn_aggr -> AllReduce -> (optional half fold) -> scale/shift [128,1]."""
    st2 = sp.tile([128, 2], F32, tag="st2" + tg)
    nc.vector.bn_aggr(st2[:], stats6[:])
    pay = sp.tile([128, 2], F32, tag="pay" + tg)
    nc.vector.tensor_copy(pay[:, 0:1], st2[:, 0:1])
    nc.vector.tensor_tensor(pay[:, 1:2], st2[:, 0:1], st2[:, 0:1], ALU.mult)
    nc.vector.tensor_tensor(pay[:, 1:2], pay[:, 1:2], st2[:, 1:2], ALU.add)
    nc.sync.dma_start(cc_in[:], pay[:])
    nc.gpsimd.collective_compute(
        "AllReduce", ALU.add, ins=[cc_in[:]], outs=[cc_out[:]], replica_groups=RG)
    tot = sp.tile([128, 2], F32, tag="tot" + tg)
    nc.sync.dma_start(tot[:], cc_out[:])
    if fold_halves:
        sw = sp.tile([128, 2], F32, tag="sw" + tg)
        nc.sync.dma_start(sw[0:64, :], cc_out[64:128, :])
        nc.sync.dma_start(sw[64:128, :], cc_out[0:64, :])
        nc.vector.tensor_tensor(tot[:], tot[:], sw[:], ALU.add)
    gm = sp.tile([128, 1], F32, tag="gm" + tg)
    nc.vector.tensor_scalar_mul(gm[:], tot[:, 0:1], 1.0 / n_shards)
    gv = sp.tile([128, 1], F32, tag="gv" + tg)
    nc.vector.tensor_scalar_mul(gv[:], tot[:, 1:2], 1.0 / n_shards)
    gm2 = sp.tile([128, 1], F32, tag="gm2" + tg)
    nc.vector.tensor_tensor(gm2[:], gm[:], gm[:], ALU.mult)
    nc.vector.tensor_tensor(gv[:], gv[:], gm2[:], ALU.subtract)
    epst = sp.tile([128, 1], F32, tag="eps" + tg)
    nc.gpsimd.memset(epst[:], EPS)
    sd = sp.tile([128, 1], F32, tag="sd" + tg)
    nc.scalar.activation(sd[:], gv[:], AF.Sqrt, bias=epst[:])
    inv = sp.tile([128, 1], F32, tag="inv" + tg)
    nc.vector.reciprocal(inv[:], sd[:])
    scale = sp.tile([128, 1], F32, tag="scale" + tg)
    nc.vector.tensor_tensor(scale[:], inv[:], bng_s[:, gcol:gcol + 1], ALU.mult)
    tmp = sp.tile([128, 1], F32, tag="tmp" + tg)
    nc.vector.tensor_tensor(tmp[:], gm[:], scale[:], ALU.mult)
    shift = sp.tile([128, 1], F32, tag="shift" + tg)
    nc.vector.tensor_tensor(shift[:], bng_s[:, bcol:bcol + 1], tmp[:], ALU.subtract)
    return scale, shift


def _trace(nc, tc, d):
    x_d = d["x_d"]

    def popen(name, bufs, space="SBUF"):
        p = tc.alloc_tile_pool(name=name, bufs=bufs, space=space)
        return p, p

    def pclose(*pools):
        for p in pools:
            p.release()

    # Pool discipline: release is strict LIFO. "Permanent" pools (small or
    # suffix-lived) open just-in-time and close only at the very end; big
    # phase temporaries live in nested scopes.
    const_cm, const = popen("const", 1)
    statp_cm, statp = popen("stats", 1)
    p1p_cm, p1p = popen("p1p", 1)
    icp_cm, icp = popen("icp", 1)
    y1p_cm, y1p = popen("y1p", 1)
    pl1_cm, pl1 = popen("pl1", 1)
    w1t_s = const.tile([27, 64], BF)
    nc.sync.dma_start(w1t_s[:], d["w1t_d"][:])
    w2t_s = const.tile([128, 9, 128], BF)
    nc.sync.dma_start(w2t_s[:], d["w2t_d"][:])
    cbs_s = const.tile([128, 16], F32)
    nc.sync.dma_start(cbs_s[:], d["cbs_d"][:])
    bng_s = const.tile([128, 4], F32)
    nc.sync.dma_start(bng_s[:], d["bng_d"][:])
    ident = const.tile([128, 128], F32)
    make_identity(nc, ident[:])

    # ------------------------------------------------------------- conv1

    ps1_cm, ps1 = popen("ps1", 6, "PSUM")
    y1 = y1p.tile([128, 8, 64, 64], BF)
    ics = [icp.tile([27, 4, 64, 64], BF, tag=f"ic{i}", name=f"ic{i}") for i in range(2)]

    stats6_1 = statp.tile([128, 64, 6], F32, tag="s61")
    t2_p1 = pl1.tile([128, 8, 32, 32], BF, tag="t2")

    for r in range(4):
        ic = ics[r % 2]
        nc.sync.dma_start(ic[0:27, 0:2], x_d[:, 2 * r:2 * r + 2])
        nc.sync.dma_start(ic[0:27, 2:4], x_d[:, 8 + 2 * r:8 + 2 * r + 2])
        for bs in range(2):
            for hj in range(8):
                pst = ps1.tile([128, 512], F32, tag="pa")
                rl = ic[0:27, bs, 8 * hj:8 * hj + 8, :]
                ru = ic[0:27, 2 + bs, 8 * hj:8 * hj + 8, :]
                nc.tensor.matmul(pst[0:64, :], w1t_s[:], rl, start=True, stop=True)
                nc.tensor.matmul(pst[64:128, :], w1t_s[:], ru, start=True, stop=True,
                                 tile_position=(0, 64))
                nc.scalar.copy(y1[:, 2 * r + bs, 8 * hj:8 * hj + 8, :],
                               pst[:].rearrange("p (h w) -> p h w", h=8))
                ci = 16 * r + 8 * bs + hj
                nc.vector.bn_stats(stats6_1[:, ci, :], pst[:])
        if r in (1, 3):
            b4 = 0 if r == 1 else 4
            t1 = pl1.tile([128, 4, 64, 32], BF, tag="t1", name=f"t1_{b4}")
            ys = y1[:, b4:b4 + 4]
            nc.vector.tensor_tensor(t1[:, :, :, 1:32], ys[:, :, :, 1:63:2], ys[:, :, :, 2:64:2], ALU.max)
            nc.vector.tensor_tensor(t1[:, :, :, 1:32], t1[:, :, :, 1:32], ys[:, :, :, 3:64:2], ALU.max)
            nc.vector.tensor_tensor(t1[:, :, :, 0:1], ys[:, :, :, 0:1], ys[:, :, :, 1:2], ALU.max)
            ts = t2_p1[:, b4:b4 + 4]
            nc.vector.tensor_tensor(ts[:, :, 1:32, :], t1[:, :, 1:63:2, :], t1[:, :, 2:64:2, :], ALU.max)
            nc.vector.tensor_tensor(ts[:, :, 1:32, :], ts[:, :, 1:32, :], t1[:, :, 3:64:2, :], ALU.max)
            nc.vector.tensor_tensor(ts[:, :, 0:1, :], t1[:, :, 0:1, :], t1[:, :, 1:2, :], ALU.max)

    scale1, shift1 = _bn_reduce_apply(
        nc, statp, stats6_1, bng_s, d["cc1_in"], d["cc1_out"],
        n_shards=16, fold_halves=True, gcol=0, bcol=1, tg="1")

    # pool1 (raw, per batch) then affine+relu into p1
    p1 = p1p.tile([128, 8, 34, 34], BF)
    nc.gpsimd.memset(p1[:, :, 0:34:33, :], 0.0)
    nc.gpsimd.memset(p1[:, :, 1:33, 0:34:33], 0.0)
    nc.scalar.activation(p1[:, :, 1:33, 1:33], t2_p1[:], AF.Relu,
                         bias=shift1[:], scale=scale1[:])

    pclose(ps1_cm)
    pclose(pl1_cm, y1p_cm, icp_cm)
    p2p_cm, p2p = popen("p2p", 1)
    pl2_cm, pl2 = popen("pl2", 1)
    y2p_cm, y2p = popen("y2p", 1)

    # ------------------------------------------------------------- conv2

    psa_cm, ps2a = popen("ps2a", 4, "PSUM")
    psb_cm, ps2b = popen("ps2b", 4, "PSUM")
    y2 = y2p.tile([128, 16, 32, 32], BF)
    stats6_2 = statp.tile([128, 32, 6], F32, tag="s62")
    t2_p2 = pl2.tile([128, 16, 16, 16], BF, tag="t2b")

    def pool2_chunk(bsl):
        t1 = pl2.tile([128, 4, 32, 16], BF, tag="t1b", name=f"t1b{bsl.start}")
        ys = y2[:, bsl]
        nc.vector.tensor_tensor(t1[:, :, :, 1:16], ys[:, :, :, 1:31:2], ys[:, :, :, 2:32:2], ALU.max)
        nc.vector.tensor_tensor(t1[:, :, :, 1:16], t1[:, :, :, 1:16], ys[:, :, :, 3:32:2], ALU.max)
        nc.vector.tensor_tensor(t1[:, :, :, 0:1], ys[:, :, :, 0:1], ys[:, :, :, 1:2], ALU.max)
        ts = t2_p2[:, bsl]
        nc.vector.tensor_tensor(ts[:, :, 1:16, :], t1[:, :, 1:31:2, :], t1[:, :, 2:32:2, :], ALU.max)
        nc.vector.tensor_tensor(ts[:, :, 1:16, :], ts[:, :, 1:16, :], t1[:, :, 3:32:2, :], ALU.max)
        nc.vector.tensor_tensor(ts[:, :, 0:1, :], t1[:, :, 0:1, :], t1[:, :, 1:2, :], ALU.max)

    for bb in range(8):
        for hj in range(2):
            pa = ps2a.tile([128, 512], F32, tag="pa2")
            pb = ps2b.tile([128, 512], F32, tag="pb2")
            for k, (dh, dw) in enumerate(OFF9):
                st, sp_ = (k == 0), (k == 8)
                rl = p1[0:64, bb, 16 * hj + dh:16 * hj + dh + 16, dw:dw + 32]
                ru = p1[64:128, bb, 16 * hj + dh:16 * hj + dh + 16, dw:dw + 32]
                nc.tensor.matmul(pa[:], w2t_s[0:64, k, :], rl, start=st, stop=sp_)
                nc.tensor.matmul(pb[:], w2t_s[64:128, k, :], ru, start=st, stop=sp_)
            for half, pp in ((0, pa), (1, pb)):
                bg = bb + 8 * half
                nc.scalar.copy(y2[:, bg, 16 * hj:16 * hj + 16, :],
                               pp[:].rearrange("p (h w) -> p h w", h=16))
                nc.vector.bn_stats(stats6_2[:, 2 * bg + hj, :], pp[:])
        if bb % 4 == 3:
            pool2_chunk(slice(bb - 3, bb + 1))
            pool2_chunk(slice(8 + bb - 3, 8 + bb + 1))

    scale2, shift2 = _bn_reduce_apply(
        nc, statp, stats6_2, bng_s, d["cc2_in"], d["cc2_out"],
        n_shards=8, fold_halves=False, gcol=2, bcol=3, tg="2")

    p2 = p2p.tile([128, 16, 18, 18], BF)
    nc.gpsimd.memset(p2[:, :, 0:18:17, :], 0.0)
    nc.gpsimd.memset(p2[:, :, 1:17, 0:18:17], 0.0)
    nc.scalar.activation(p2[:, :, 1:17, 1:17], t2_p2[:], AF.Relu,
                         bias=shift2[:], scale=scale2[:])

    pclose(psb_cm, psa_cm)
    pclose(y2p_cm, pl2_cm)
    p3p_cm, p3p = popen("p3p", 1)
    w3p_cm, w3p = popen("w3p", 1)
    y3p_cm, y3p = popen("y3p", 1)

    # ------------------------------------------------------------- conv3

    psc_cm, psc = popen("psc", 4, "PSUM")
    w3t_s = w3p.tile([128, 2, 1, 9, 128], BF)
    for ct in range(2):
        nc.sync.dma_start(w3t_s[:, ct], d["w3t_d"][ct])
    y3 = y3p.tile([128, 2, 16, 18, 18], BF)
    nc.gpsimd.memset(y3[:, :, :, 0:18:17, :], 0.0)
    nc.gpsimd.memset(y3[:, :, :, 1:17, 0:18:17], 0.0)
    for ct in range(2):
        for bp in range(8):
            pst = psc.tile([128, 512], F32, tag="pc")
            for k, (dh, dw) in enumerate(OFF9):
                rhs = p2[:, 2 * bp:2 * bp + 2, dh:dh + 16, dw:dw + 16]
                nc.tensor.matmul(pst[:], w3t_s[:, ct, 0, k, :], rhs,
                                 start=(k == 0), stop=(k == 8))
            nc.scalar.activation(
                y3[:, ct, 2 * bp:2 * bp + 2, 1:17, 1:17],
                pst[:].rearrange("p (b h w) -> p b h w", b=2, h=16),
                AF.Relu, bias=cbs_s[:, ct:ct + 1])

    # --------------------------------------------------------- conv4

    w4p_cm, w4p = popen("w4p", 1)
    y4p_cm, y4p = popen("y4p", 1)
    pl3_cm, pl3 = popen("pl3", 2)
    w4t_s = w4p.tile([128, 2, 2, 9, 128], BF)
    for ct in range(2):
        nc.sync.dma_start(w4t_s[:, ct], d["w4t_d"][ct])
    y4r = y4p.tile([128, 2, 16, 18, 18], BF)
    nc.gpsimd.memset(y4r[:, :, :, 0:18:17, :], NEG)
    nc.gpsimd.memset(y4r[:, :, :, 1:17, 0:18:17], NEG)
    for ct in range(2):
        for bp in range(8):
            pst = psc.tile([128, 512], F32, tag="pc")
            first = True
            for kt in range(2):
                for k, (dh, dw) in enumerate(OFF9):
                    rhs = y3[:, kt, 2 * bp:2 * bp + 2, dh:dh + 16, dw:dw + 16]
                    nc.tensor.matmul(pst[:], w4t_s[:, ct, kt, k, :], rhs,
                                     start=first, stop=(kt == 1 and k == 8))
                    first = False
            nc.scalar.copy(
                y4r[:, ct, 2 * bp:2 * bp + 2, 1:17, 1:17],
                pst[:].rearrange("p (b h w) -> p b h w", b=2, h=16))

    # pool3 + bias+relu -> p3 [128, 2, 16, 10, 10]
    p3 = p3p.tile([128, 2, 16, 10, 10], BF)
    nc.gpsimd.memset(p3[:, :, :, 0:10:9, :], 0.0)
    nc.gpsimd.memset(p3[:, :, :, 1:9, 0:10:9], 0.0)
    for ct in range(2):
        t1 = pl3.tile([128, 16, 18, 8], BF, tag="t1c")
        nc.vector.tensor_tensor(t1[:], y4r[:, ct, :, :, 0:16:2],
                                y4r[:, ct, :, :, 1:17:2], ALU.max)
        nc.vector.tensor_tensor(t1[:], t1[:], y4r[:, ct, :, :, 2:18:2], ALU.max)
        t2 = pl3.tile([128, 16, 8, 8], BF, tag="t2c")
        nc.vector.tensor_tensor(t2[:], t1[:, :, 0:16:2, :], t1[:, :, 1:17:2, :], ALU.max)
        nc.vector.tensor_tensor(t2[:], t2[:], t1[:, :, 2:18:2, :], ALU.max)
        nc.scalar.activation(p3[:, ct, :, 1:9, 1:9], t2[:], AF.Relu,
                             bias=cbs_s[:, 2 + ct:3 + ct])

    pclose(pl3_cm, y4p_cm, w4p_cm, y3p_cm, w3p_cm)
    p4p_cm, p4p = popen("p4p", 1)
    y7p_cm, y7p_pool = popen("y7p", 1)
    y5p_cm, y5p = popen("y5p", 1)
    wstg_cm, wstg = popen("wstg", 2)
    pl4_cm, pl4 = popen("pl4", 2)

    # ----------------------------------------------------- conv5

    y5 = y5p.tile([128, 4, 16, 10, 10], BF, tag="y5")
    nc.gpsimd.memset(y5[:, :, :, 0:10:9, :], 0.0)
    nc.gpsimd.memset(y5[:, :, :, 1:9, 0:10:9], 0.0)
    for ct in range(4):
        w5c = wstg.tile([128, 2, 9, 128], BF, tag="wc5", name=f"w5c{ct}")
        nc.sync.dma_start(w5c[:], d["w5t_d"][ct])
        for bp in range(2):
            pst = psc.tile([128, 512], F32, tag="pc")
            first = True
            for kt in range(2):
                for k, (dh, dw) in enumerate(OFF9):
                    rhs = p3[:, kt, 8 * bp:8 * bp + 8, dh:dh + 8, dw:dw + 8]
                    nc.tensor.matmul(pst[:], w5c[:, kt, k, :], rhs,
                                     start=first, stop=(kt == 1 and k == 8))
                    first = False
            nc.scalar.activation(
                y5[:, ct, 8 * bp:8 * bp + 8, 1:9, 1:9],
                pst[:].rearrange("p (b h w) -> p b h w", b=8, h=8),
                AF.Relu, bias=cbs_s[:, 4 + ct:5 + ct])

    # ------------------------------------------------- conv6

    y6r = y5p.tile([128, 4, 16, 10, 10], BF, tag="y6r")
    nc.gpsimd.memset(y6r[:, :, :, 0:10:9, :], NEG)
    nc.gpsimd.memset(y6r[:, :, :, 1:9, 0:10:9], NEG)
    for ct in range(4):
        w6c = wstg.tile([128, 4, 9, 128], BF, tag="wc6", name=f"w6c{ct}")
        nc.sync.dma_start(w6c[:], d["w6t_d"][ct])
        for bp in range(2):
            pst = psc.tile([128, 512], F32, tag="pc")
            first = True
            for kt in range(4):
                for k, (dh, dw) in enumerate(OFF9):
                    rhs = y5[:, kt, 8 * bp:8 * bp + 8, dh:dh + 8, dw:dw + 8]
                    nc.tensor.matmul(pst[:], w6c[:, kt, k, :], rhs,
                                     start=first, stop=(kt == 3 and k == 8))
                    first = False
            nc.scalar.copy(
                y6r[:, ct, 8 * bp:8 * bp + 8, 1:9, 1:9],
                pst[:].rearrange("p (b h w) -> p b h w", b=8, h=8))
    # pool4 + bias+relu -> p4 [128, 4, 16, 6, 6]
    p4 = p4p.tile([128, 4, 16, 6, 6], BF)
    nc.gpsimd.memset(p4[:, :, :, 0:6:5, :], 0.0)
    nc.gpsimd.memset(p4[:, :, :, 1:5, 0:6:5], 0.0)
    for ct in range(4):
        t1 = pl4.tile([128, 16, 10, 4], BF, tag="t1d")
        nc.vector.tensor_tensor(t1[:], y6r[:, ct, :, :, 0:8:2],
                                y6r[:, ct, :, :, 1:9:2], ALU.max)
        nc.vector.tensor_tensor(t1[:], t1[:], y6r[:, ct, :, :, 2:10:2], ALU.max)
        t2 = pl4.tile([128, 16, 4, 4], BF, tag="t2d")
        nc.vector.tensor_tensor(t2[:], t1[:, :, 0:8:2, :], t1[:, :, 1:9:2, :], ALU.max)
        nc.vector.tensor_tensor(t2[:], t2[:], t1[:, :, 2:10:2, :], ALU.max)
        nc.scalar.activation(p4[:, ct, :, 1:5, 1:5], t2[:], AF.Relu,
                             bias=cbs_s[:, 8 + ct:9 + ct])

    pclose(pl4_cm)


    # ------------------------------------------------- conv7
    y7 = y7p_pool.tile([128, 4, 16, 16], BF, name="y7")
    for ct in range(4):
        w7c = wstg.tile([128, 4, 9, 128], BF, tag="wc6", name=f"w7c{ct}")
        nc.sync.dma_start(w7c[:], d["w7t_d"][ct])
        pst = psc.tile([128, 256], F32, tag="pc7")
        first = True
        for kt in range(4):
            for k, (dh, dw) in enumerate(OFF9):
                rhs = p4[:, kt, :, dh:dh + 4, dw:dw + 4]
                nc.tensor.matmul(pst[:], w7c[:, kt, k, :], rhs,
                                 start=first, stop=(kt == 3 and k == 8))
                first = False
        nc.scalar.activation(
            y7[:, ct, :, :],
            pst[:].rearrange("p (b n) -> p b n", b=16),
            AF.Relu, bias=cbs_s[:, 12 + ct:13 + ct])

    pclose(psc_cm)
    pclose(wstg_cm, y5p_cm)

    # --------------------------------------------- colons

    colp_cm, colp = popen("colp", 1)
    hp_cm, hp = popen("hp", 3)
    sxp_cm, sxp = popen("sxp", 4)
    colw_cm, colw = popen("colw", 1)
    psU_cm, psU = popen("psU", 3, "PSUM")
    psL_cm, psL = popen("psL", 3, "PSUM")
    psT_cm, psT = popen("psT", 1, "PSUM")
    wnp_cm, wnp = popen("wnp", 3)
    b1r_s = colw.tile([1, 16, 5, 128], BF)
    nc.sync.dma_start(b1r_s[:], d["b1r_d"][:])
    w2c_s = colw.tile([128, 16, 5, 10], BF)
    nc.sync.dma_start(w2c_s[:], d["w2c_d"][:])
    b2r_s = colw.tile([1, 16, 10], BF)
    nc.sync.dma_start(b2r_s[:], d["b2r_d"][:])
    ones1 = colw.tile([1, 16], BF)
    nc.gpsimd.memset(ones1[:], 1.0)

    U_sb = colp.tile([128, 16, 5, 16], F32)
    h1 = colp.tile([128, 16, 5, 16], BF)
    preds1 = colp.tile([16, 16, 10], F32)
    preds2 = colp.tile([16, 16, 10], F32)

    def colon_tail(n, h_t, logits_buf):
        L_ps = psL.tile([16, 10], F32, tag="lp")
        for kt in range(5):
            nc.tensor.matmul(L_ps[:], h_t[:, kt, :], w2c_s[:, n, kt, :],
                             start=(kt == 0), stop=False)
        nc.tensor.matmul(L_ps[:], ones1[:], b2r_s[:, n, :], start=False, stop=True)
        nc.scalar.copy(logits_buf[:, n, :], L_ps[:])

    def batched_softmax(logits_buf, preds_out):
        # softmax over the last dim for all 16 nodes in 6 ops
        mx = sxp.tile([16, 16], F32, tag="mx")
        nc.vector.tensor_reduce(mx[:], logits_buf[:], AX.X, ALU.max, negate=True)
        sh = sxp.tile([16, 16, 10], F32, tag="sh")
        nc.vector.tensor_tensor(sh[:], logits_buf[:],
                                mx[:, :, None].to_broadcast((16, 16, 10)), ALU.add)
        ex = sxp.tile([16, 16, 10], F32, tag="ex")
        nc.scalar.activation(ex[:], sh[:], AF.Exp)
        sm = sxp.tile([16, 16], F32, tag="sm")
        nc.vector.tensor_reduce(sm[:], ex[:], AX.X, ALU.add)
        rc = sxp.tile([16, 16], F32, tag="rc")
        nc.vector.reciprocal(rc[:], sm[:])
        nc.vector.tensor_tensor(preds_out[:], ex[:],
                                rc[:, :, None].to_broadcast((16, 16, 10)), ALU.mult)

    # pass 1
    logits1 = colp.tile([16, 16, 10], F32)
    logits2 = colp.tile([16, 16, 10], F32)
    for n in range(16):
        w1n = wnp.tile([128, 4, 5, 128], BF, tag="w1n", name=f"w1n{n}")
        nc.sync.dma_start(w1n[:], d["w1f_d"][:, n])
        up = psU.tile([128, 5, 16], F32, tag="up")
        for mt in range(5):
            for kt in range(4):
                nc.tensor.matmul(up[:, mt, :], w1n[:, kt, mt, :], y7[:, kt, :, n],
                                 start=(kt == 0), stop=False)
            nc.tensor.matmul(up[:, mt, :], b1r_s[:, n, mt, :], ones1[:],
                             start=False, stop=(mt == 4))
        nc.vector.tensor_copy(U_sb[:, n, :, :], up[:])
        nc.scalar.activation(h1[:, n, :, :], up[:], AF.Tanh)
        colon_tail(n, h1[:, n, :, :], logits1)
    batched_softmax(logits1, preds1)

    # transpose preds1 -> predsT [128|32, 16]
    pv = preds1[:].rearrange("p n c -> p (n c)")
    predsT = colp.tile([128, 2, 16], BF)
    nc.gpsimd.memset(predsT[:, 1, :], 0.0)
    ptp = psT.tile([128, 16], F32, tag="ptp")
    nc.tensor.transpose(ptp[:], pv[:, 0:128], ident[0:16, 0:16])
    nc.scalar.copy(predsT[:, 0, :], ptp[:])
    ptp2 = psT.tile([32, 16], F32, tag="ptp2")
    nc.tensor.transpose(ptp2[:], pv[:, 128:160], ident[0:16, 0:16])
    nc.scalar.copy(predsT[0:32, 1, :], ptp2[:])

    # pass 2
    for n in range(16):
        wtn = wnp.tile([128, 2, 5, 128], BF, tag="wtn", name=f"wtn{n}")
        nc.sync.dma_start(wtn[:], d["wt_d"][:, n])
        h2 = hp.tile([128, 5, 16], BF, tag="h2")
        dp = psU.tile([128, 5, 16], F32, tag="up")
        for mt in range(5):
            for kt in range(2):
                nc.tensor.matmul(dp[:, mt, :], wtn[:, kt, mt, :], predsT[:, kt, :],
                                 start=(kt == 0), stop=(kt == 1 and mt == 4))
        vv = sxp.tile([128, 5, 16], F32, tag="vv")
        nc.vector.tensor_tensor(vv[:], U_sb[:, n, :, :], dp[:], ALU.add)
        nc.scalar.activation(h2[:], vv[:], AF.Tanh)
        colon_tail(n, h2[:], logits2)
    batched_softmax(logits2, preds2)

    nc.sync.dma_start(d["out_d"][:], preds2[:])

    pclose(psT_cm, psL_cm, psU_cm)
    pclose(wnp_cm, colw_cm, sxp_cm, hp_cm, colp_cm, y7p_cm, p4p_cm, p3p_cm,
           p2p_cm, p1p_cm, statp_cm, const_cm)


_NC_CACHE = None


def kernel(**inputs):
    global _NC_CACHE, _LAST_RESULT
    g = _prep(inputs)
    xi_full = g.pop("_xi_full")

    if _NC_CACHE is None:
        _NC_CACHE = _build()
    nc = _NC_CACHE

    in_maps = []
    for c in range(N_CORES):
        m = {"xi": np.ascontiguousarray(xi_full[:, c * BL:(c + 1) * BL])}
        m.update(g)
        in_maps.append(m)

    res = bass_utils.run_bass_kernel_spmd(nc, in_maps, core_ids=list(range(N_CORES)))
    _LAST_RESULT = res

    preds_all = np.concatenate([res.results[c]["preds2"] for c in range(N_CORES)], axis=0)
    preds2 = np.ascontiguousarray(preds_all.transpose(1, 0, 2)).astype(np.float32)
    sum_preds = preds2.mean(axis=0)
    return (sum_preds * sum_preds).astype(np.float32), preds2


# revision 25
# speedup vs baseline: 1.0108x; 1.0108x over previous
"""Trainium2 Bass kernel for nn_Brain (7-conv CNN backbone + BN(train) +
16 per-node MLP colons with one message-passing round), data-parallel over
batch across 8 NeuronCores.

Self-contained: hardcodes all shapes/sharding. Returns (squared_sum_preds,
preds2) like the reference.
"""
import numpy as np
import ml_dtypes

import concourse.bass as bass
import concourse.mybir as mybir
import concourse.tile as tile
from concourse import bass_utils
from concourse.masks import make_identity
from concourse.vector_clock import ScopedClock

bf16 = ml_dtypes.bfloat16
F32 = mybir.dt.float32
BF = mybir.dt.bfloat16
AF = mybir.ActivationFunctionType
ALU = mybir.AluOpType
AX = mybir.AxisListType

N_CORES = 8
B = 128
BL = 16            # batch per core
NEG = -1.0e30      # -inf-ish padding for raw max-pools
EPS = 1e-5
OFF9 = [(dh, dw) for dh in range(3) for dw in range(3)]
RG = [list(range(N_CORES))]

_LAST_RESULT = None


# ---------------------------------------------------------------------------
# compat TileContext: this container's walrus accepts at most ONE sync wait
# per instruction; split extra waits onto NOPs on the same engine.
_REAL_ENGINES = {
    mybir.EngineType.PE,
    mybir.EngineType.DVE,
    mybir.EngineType.Activation,
    mybir.EngineType.Pool,
    mybir.EngineType.SP,
}


class CompatTileContext(tile.TileContext):
    MAX_SYNC = 1

    def _commit_instruction(self, inst, lazy_reg_writes=True):
        si = getattr(inst, "sync_info", None)
        if (
            si is not None
            and si.on_wait
            and len(si.on_wait) > 1
            and inst.engine in _REAL_ENGINES
        ):
            waits = list(si.on_wait)
            si.on_wait = waits[-1:]
            for k, w in enumerate(waits[:-1]):
                nop = mybir.InstNoOp(
                    name=f"{inst.name}-xw{k}",
                    sync_info=mybir.SyncInfo(on_wait=[w], on_update=[]),
                    bass_nofuse=True,
                    engine=inst.engine,
                )
                super()._commit_instruction(nop, lazy_reg_writes=False)
        return super()._commit_instruction(inst, lazy_reg_writes)

    def _drain_and_barrier(self, tick_clock, wait_clock):
        nop0 = self.nc.sync.nop(nofuse=True)
        wait_clock.add_sem_waits(nop0.ins, ScopedClock({None: tick_clock.global_clock}))
        si = nop0.ins.sync_info
        waits = list(si.on_wait) if si is not None and si.on_wait else []
        if len(waits) > self.MAX_SYNC:
            si.on_wait = waits[: self.MAX_SYNC]
            rest = waits[self.MAX_SYNC:]
            while rest:
                nop = self.nc.sync.nop(nofuse=True)
                nsi = nop.ins.sync_info
                chunk, rest = rest[: self.MAX_SYNC], rest[self.MAX_SYNC:]
                if nsi is None:
                    nop.ins.sync_info = mybir.SyncInfo(on_wait=chunk, on_update=[])
                else:
                    nsi.on_wait = chunk
        self.nc.sync.drain()

        self.nc.all_engine_barrier()
        assert self.sems is not None
        popped = self.nc._tile_sem_poison_stack.pop()
        assert popped is self._sem_poison
        self.nc.clear_and_free_semaphores(list(self.sems.allocated().values()))
        self.nc.all_engine_barrier()


# ---------------------------------------------------------------------------
# host-side weight preparation

def _neighbors(i, w, h):
    size = w * h
    out = []
    if i - w >= 0:
        out.append(i - w)
    if i % w != 0:
        out.append(i - 1)
    if (i + 1) % w != 0:
        out.append(i + 1)
    if i + w < size:
        out.append(i + w)
    if i - w - 1 >= 0 and i % w != 0:
        out.append(i - w - 1)
    if i - w + 1 >= 0 and (i + 1) % w != 0:
        out.append(i - w + 1)
    if i + w - 1 < size and i % w != 0:
        out.append(i + w - 1)
    if i + w + 1 < size and (i + 1) % w != 0:
        out.append(i + w + 1)
    return out


def _prep(inputs):
    f = np.float32
    g = {}

    cw1 = np.asarray(inputs["cw1"], f)
    g["w1t"] = np.ascontiguousarray(cw1.transpose(2, 3, 1, 0).reshape(27, 64)).astype(bf16)

    x = np.asarray(inputs["x"], f).astype(bf16)        # [128, 3, 64, 64]
    xt = x.transpose(1, 0, 2, 3)                        # [3, 128, 64, 64]
    xi = np.zeros((27, 128, 64, 64), bf16)
    for k, (dh, dw) in enumerate([(a, b) for a in range(3) for b in range(3)]):
        hs, he = max(0, 1 - dh), 64 + min(0, 1 - dh)
        ws, we = max(0, 1 - dw), 64 + min(0, 1 - dw)
        xi[3 * k:3 * k + 3, :, hs:he, ws:we] = \
            xt[:, :, hs + dh - 1:he + dh - 1, ws + dw - 1:we + dw - 1]
    g["_xi_full"] = xi

    w2 = np.asarray(inputs["cw2"], f).transpose(2, 3, 1, 0).reshape(9, 64, 128)
    g["w2t"] = np.ascontiguousarray(
        np.concatenate([w2, w2], axis=1).transpose(1, 0, 2)).astype(bf16)  # [128,9,128]

    def conv_t(cw, kt, ct):
        a = np.asarray(cw, f).transpose(2, 3, 1, 0).reshape(9, kt, 128, ct, 128)
        return np.ascontiguousarray(a.transpose(3, 2, 1, 0, 4)).astype(bf16)  # [ct,p,kt,k,q]

    g["w3t"] = conv_t(inputs["cw3"], 1, 2)
    g["w4t"] = conv_t(inputs["cw4"], 2, 2)
    g["w5t"] = conv_t(inputs["cw5"], 2, 4)
    g["w6t"] = conv_t(inputs["cw6"], 4, 4)
    g["w7t"] = conv_t(inputs["cw7"], 4, 4)

    cbs = np.zeros((128, 16), f)
    cbs[:, 0:2] = np.asarray(inputs["cb3"], f).reshape(2, 128).T
    cbs[:, 2:4] = np.asarray(inputs["cb4"], f).reshape(2, 128).T
    cbs[:, 4:8] = np.asarray(inputs["cb5"], f).reshape(4, 128).T
    cbs[:, 8:12] = np.asarray(inputs["cb6"], f).reshape(4, 128).T
    cbs[:, 12:16] = np.asarray(inputs["cb7"], f).reshape(4, 128).T
    g["cbs"] = cbs

    bng = np.zeros((128, 4), f)
    bng[:, 0] = np.tile(np.asarray(inputs["bn1_g"], f), 2)
    bng[:, 1] = np.tile(np.asarray(inputs["bn1_b"], f), 2)
    bng[:, 2] = np.asarray(inputs["bn2_g"], f)
    bng[:, 3] = np.asarray(inputs["bn2_b"], f)
    g["bng"] = bng

    W1 = np.asarray(inputs["W1"], f)        # [16, 592, 600]
    b1 = np.asarray(inputs["b1"], f)        # [16, 600]
    W2 = np.asarray(inputs["W2"], f)        # [16, 600, 10]
    b2 = np.asarray(inputs["b2"], f)        # [16, 10]

    w1f = np.zeros((16, 512, 640), f)
    w1f[:, :, :600] = W1[:, 80:, :]
    g["w1f"] = np.ascontiguousarray(
        w1f.reshape(16, 4, 128, 5, 128).transpose(2, 0, 1, 3, 4)).astype(bf16)

    idxs = np.zeros((16, 8), np.int64)
    mask = np.zeros((16, 8), f)
    for i in range(16):
        nb = _neighbors(i, 4, 4)
        idxs[i, :len(nb)] = nb
        mask[i, :len(nb)] = 1.0
    wt = np.zeros((16, 256, 640), f)
    for n in range(16):
        G = np.zeros((80, 160), f)
        for j in range(8):
            if mask[n, j] > 0:
                for c in range(10):
                    G[10 * j + c, 10 * idxs[n, j] + c] = 1.0
        wt[n, :160, :600] = G.T @ W1[n, :80, :]
    g["wt"] = np.ascontiguousarray(
        wt.reshape(16, 2, 128, 5, 128).transpose(2, 0, 1, 3, 4)).astype(bf16)
    g["b2r"] = np.ascontiguousarray(b2.reshape(1, 16, 10)).astype(bf16)

    b1p = np.zeros((16, 640), f)
    b1p[:, :600] = b1
    g["b1r"] = np.ascontiguousarray(b1p.reshape(1, 16, 5, 128)).astype(bf16)

    w2p = np.zeros((16, 640, 10), f)
    w2p[:, :600] = W2
    g["w2c"] = np.ascontiguousarray(
        w2p.reshape(16, 5, 128, 10).transpose(2, 0, 1, 3)).astype(bf16)
    return g


# ---------------------------------------------------------------------------
# device program

def _build():
    nc = bass.Bass()
    d = {}
    d["x_d"] = nc.dram_tensor("xi", [27, BL, 64, 64], BF, kind="ExternalInput")
    d["w1t_d"] = nc.dram_tensor("w1t", [27, 64], BF, kind="ExternalInput")
    d["w2t_d"] = nc.dram_tensor("w2t", [128, 9, 128], BF, kind="ExternalInput")
    d["w3t_d"] = nc.dram_tensor("w3t", [2, 128, 1, 9, 128], BF, kind="ExternalInput")
    d["w4t_d"] = nc.dram_tensor("w4t", [2, 128, 2, 9, 128], BF, kind="ExternalInput")
    d["w5t_d"] = nc.dram_tensor("w5t", [4, 128, 2, 9, 128], BF, kind="ExternalInput")
    d["w6t_d"] = nc.dram_tensor("w6t", [4, 128, 4, 9, 128], BF, kind="ExternalInput")
    d["w7t_d"] = nc.dram_tensor("w7t", [4, 128, 4, 9, 128], BF, kind="ExternalInput")
    d["cbs_d"] = nc.dram_tensor("cbs", [128, 16], F32, kind="ExternalInput")
    d["bng_d"] = nc.dram_tensor("bng", [128, 4], F32, kind="ExternalInput")
    d["w1f_d"] = nc.dram_tensor("w1f", [128, 16, 4, 5, 128], BF, kind="ExternalInput")
    d["wt_d"] = nc.dram_tensor("wt", [128, 16, 2, 5, 128], BF, kind="ExternalInput")
    d["b1r_d"] = nc.dram_tensor("b1r", [1, 16, 5, 128], BF, kind="ExternalInput")
    d["w2c_d"] = nc.dram_tensor("w2c", [128, 16, 5, 10], BF, kind="ExternalInput")
    d["b2r_d"] = nc.dram_tensor("b2r", [1, 16, 10], BF, kind="ExternalInput")
    d["out_d"] = nc.dram_tensor("preds2", [BL, 16, 10], F32, kind="ExternalOutput")

    d["cc1_in"] = nc.dram_tensor("cc1_in", [128, 2], F32)
    d["cc1_out"] = nc.dram_tensor("cc1_out", [128, 2], F32, addr_space="Shared")
    d["cc2_in"] = nc.dram_tensor("cc2_in", [128, 2], F32)
    d["cc2_out"] = nc.dram_tensor("cc2_out", [128, 2], F32, addr_space="Shared")

    with CompatTileContext(nc, pool_alloc_mode="queue") as tc:
        _trace(nc, tc, d)
    return nc


def _bn_reduce_apply(nc, sp, stats6, bng_s, cc_in, cc_out, n_shards, fold_halves,
                     gcol, bcol, tg):
    """bn_aggr -> AllReduce -> (optional half fold) -> scale/shift [128,1]."""
    st2 = sp.tile([128, 2], F32, tag="st2" + tg)
    nc.vector.bn_aggr(st2[:], stats6[:])
    pay = sp.tile([128, 2], F32, tag="pay" + tg)
    nc.vector.tensor_copy(pay[:, 0:1], st2[:, 0:1])
    nc.vector.tensor_tensor(pay[:, 1:2], st2[:, 0:1], st2[:, 0:1], ALU.mult)
    nc.vector.tensor_tensor(pay[:, 1:2], pay[:, 1:2], st2[:, 1:2], ALU.add)
    nc.sync.dma_start(cc_in[:], pay[:])
    nc.gpsimd.collective_compute(
        "AllReduce", ALU.add, ins=[cc_in[:]], outs=[cc_out[:]], replica_groups=RG)
    tot = sp.tile([128, 2], F32, tag="tot" + tg)
    nc.sync.dma_start(tot[:], cc_out[:])
    if fold_halves:
        sw = sp.tile([128, 2], F32, tag="sw" + tg)
        nc.sync.dma_start(sw[0:64, :], cc_out[64:128, :])
        nc.sync.dma_start(sw[64:128, :], cc_out[0:64, :])
        nc.vector.tensor_tensor(tot[:], tot[:], sw[:], ALU.add)
    gm = sp.tile([128, 1], F32, tag="gm" + tg)
    nc.vector.tensor_scalar_mul(gm[:], tot[:, 0:1], 1.0 / n_shards)
    gv = sp.tile([128, 1], F32, tag="gv" + tg)
    nc.vector.tensor_scalar_mul(gv[:], tot[:, 1:2], 1.0 / n_shards)
    gm2 = sp.tile([128, 1], F32, tag="gm2" + tg)
    nc.vector.tensor_tensor(gm2[:], gm[:], gm[:], ALU.mult)
    nc.vector.tensor_tensor(gv[:], gv[:], gm2[:], ALU.subtract)
    epst = sp.tile([128, 1], F32, tag="eps" + tg)
    nc.gpsimd.memset(epst[:], EPS)
    sd = sp.tile([128, 1], F32, tag="sd" + tg)
    nc.scalar.activation(sd[:], gv[:], AF.Sqrt, bias=epst[:])
    inv = sp.tile([128, 1], F32, tag="inv" + tg)
    nc.vector.reciprocal(inv[:], sd[:])
    scale = sp.tile([128, 1], F32, tag="scale" + tg)
    nc.vector.tensor_tensor(scale[:], inv[:], bng_s[:, gcol:gcol + 1], ALU.mult)
    tmp = sp.tile([128, 1], F32, tag="tmp" + tg)
    nc.vector.tensor_tensor(tmp[:], gm[:], scale[:], ALU.mult)
    shift = sp.tile([128, 1], F32, tag="shift" + tg)
    nc.vector.tensor_tensor(shift[:], bng_s[:, bcol:bcol + 1], tmp[:], ALU.subtract)
    return scale, shift


def _trace(nc, tc, d):
    x_d = d["x_d"]

    def popen(name, bufs, space="SBUF"):
        p = tc.alloc_tile_pool(name=name, bufs=bufs, space=space)
        return p, p

    def pclose(*pools):
        for p in pools:
            p.release()

    # Pool discipline: release is strict LIFO. "Permanent" pools (small or
    # suffix-lived) open just-in-time and close only at the very end; big
    # phase temporaries live in nested scopes.
    const_cm, const = popen("const", 1)
    statp_cm, statp = popen("stats", 1)
    p1p_cm, p1p = popen("p1p", 1)
    icp_cm, icp = popen("icp", 1)
    y1p_cm, y1p = popen("y1p", 1)
    pl1_cm, pl1 = popen("pl1", 1)
    w1t_s = const.tile([27, 64], BF)
    nc.sync.dma_start(w1t_s[:], d["w1t_d"][:])
    w2t_s = const.tile([128, 9, 128], BF)
    nc.sync.dma_start(w2t_s[:], d["w2t_d"][:])
    cbs_s = const.tile([128, 16], F32)
    nc.sync.dma_start(cbs_s[:], d["cbs_d"][:])
    bng_s = const.tile([128, 4], F32)
    nc.sync.dma_start(bng_s[:], d["bng_d"][:])
    ident = const.tile([128, 128], F32)
    make_identity(nc, ident[:])

    # ------------------------------------------------------------- conv1

    ps1_cm, ps1 = popen("ps1", 6, "PSUM")
    y1 = y1p.tile([128, 8, 64, 64], BF)
    ics = [icp.tile([27, 4, 64, 64], BF, tag=f"ic{i}", name=f"ic{i}") for i in range(2)]

    stats6_1 = statp.tile([128, 64, 6], F32, tag="s61")
    t2_p1 = pl1.tile([128, 8, 32, 32], BF, tag="t2")

    for r in range(4):
        ic = ics[r % 2]
        nc.sync.dma_start(ic[0:27, 0:2], x_d[:, 2 * r:2 * r + 2])
        nc.sync.dma_start(ic[0:27, 2:4], x_d[:, 8 + 2 * r:8 + 2 * r + 2])
        for bs in range(2):
            for hj in range(8):
                pst = ps1.tile([128, 512], F32, tag="pa")
                rl = ic[0:27, bs, 8 * hj:8 * hj + 8, :]
                ru = ic[0:27, 2 + bs, 8 * hj:8 * hj + 8, :]
                nc.tensor.matmul(pst[0:64, :], w1t_s[:], rl, start=True, stop=True)
                nc.tensor.matmul(pst[64:128, :], w1t_s[:], ru, start=True, stop=True,
                                 tile_position=(0, 64))
                nc.scalar.copy(y1[:, 2 * r + bs, 8 * hj:8 * hj + 8, :],
                               pst[:].rearrange("p (h w) -> p h w", h=8))
                ci = 16 * r + 8 * bs + hj
                nc.vector.bn_stats(stats6_1[:, ci, :], pst[:])
        if True:
            b4 = 2 * r
            t1 = pl1.tile([128, 2, 64, 32], BF, tag="t1", name=f"t1_{b4}")
            ys = y1[:, b4:b4 + 2]
            nc.vector.tensor_tensor(t1[:, :, :, 1:32], ys[:, :, :, 1:63:2], ys[:, :, :, 2:64:2], ALU.max)
            nc.vector.tensor_tensor(t1[:, :, :, 1:32], t1[:, :, :, 1:32], ys[:, :, :, 3:64:2], ALU.max)
            nc.vector.tensor_tensor(t1[:, :, :, 0:1], ys[:, :, :, 0:1], ys[:, :, :, 1:2], ALU.max)
            ts = t2_p1[:, b4:b4 + 2]
            nc.vector.tensor_tensor(ts[:, :, 1:32, :], t1[:, :, 1:63:2, :], t1[:, :, 2:64:2, :], ALU.max)
            nc.vector.tensor_tensor(ts[:, :, 1:32, :], ts[:, :, 1:32, :], t1[:, :, 3:64:2, :], ALU.max)
            nc.vector.tensor_tensor(ts[:, :, 0:1, :], t1[:, :, 0:1, :], t1[:, :, 1:2, :], ALU.max)

    scale1, shift1 = _bn_reduce_apply(
        nc, statp, stats6_1, bng_s, d["cc1_in"], d["cc1_out"],
        n_shards=16, fold_halves=True, gcol=0, bcol=1, tg="1")

    # pool1 (raw, per batch) then affine+relu into p1
    p1 = p1p.tile([128, 8, 34, 34], BF)
    nc.gpsimd.memset(p1[:, :, 0:34:33, :], 0.0)
    nc.gpsimd.memset(p1[:, :, 1:33, 0:34:33], 0.0)
    nc.scalar.activation(p1[:, :, 1:33, 1:33], t2_p1[:], AF.Relu,
                         bias=shift1[:], scale=scale1[:])

    pclose(ps1_cm)
    pclose(pl1_cm, y1p_cm, icp_cm)
    p2p_cm, p2p = popen("p2p", 1)
    pl2_cm, pl2 = popen("pl2", 1)
    y2p_cm, y2p = popen("y2p", 1)

    # ------------------------------------------------------------- conv2

    psa_cm, ps2a = popen("ps2a", 4, "PSUM")
    psb_cm, ps2b = popen("ps2b", 4, "PSUM")
    y2 = y2p.tile([128, 16, 32, 32], BF)
    stats6_2 = statp.tile([128, 32, 6], F32, tag="s62")
    t2_p2 = pl2.tile([128, 16, 16, 16], BF, tag="t2b")

    def pool2_chunk(bsl):
        t1 = pl2.tile([128, 2, 32, 16], BF, tag="t1b", name=f"t1b{bsl.start}")
        ys = y2[:, bsl]
        nc.vector.tensor_tensor(t1[:, :, :, 1:16], ys[:, :, :, 1:31:2], ys[:, :, :, 2:32:2], ALU.max)
        nc.vector.tensor_tensor(t1[:, :, :, 1:16], t1[:, :, :, 1:16], ys[:, :, :, 3:32:2], ALU.max)
        nc.vector.tensor_tensor(t1[:, :, :, 0:1], ys[:, :, :, 0:1], ys[:, :, :, 1:2], ALU.max)
        ts = t2_p2[:, bsl]
        nc.vector.tensor_tensor(ts[:, :, 1:16, :], t1[:, :, 1:31:2, :], t1[:, :, 2:32:2, :], ALU.max)
        nc.vector.tensor_tensor(ts[:, :, 1:16, :], ts[:, :, 1:16, :], t1[:, :, 3:32:2, :], ALU.max)
        nc.vector.tensor_tensor(ts[:, :, 0:1, :], t1[:, :, 0:1, :], t1[:, :, 1:2, :], ALU.max)

    for bb in range(8):
        for hj in range(2):
            pa = ps2a.tile([128, 512], F32, tag="pa2")
            pb = ps2b.tile([128, 512], F32, tag="pb2")
            for k, (dh, dw) in enumerate(OFF9):
                st, sp_ = (k == 0), (k == 8)
                rl = p1[0:64, bb, 16 * hj + dh:16 * hj + dh + 16, dw:dw + 32]
                ru = p1[64:128, bb, 16 * hj + dh:16 * hj + dh + 16, dw:dw + 32]
                nc.tensor.matmul(pa[:], w2t_s[0:64, k, :], rl, start=st, stop=sp_)
                nc.tensor.matmul(pb[:], w2t_s[64:128, k, :], ru, start=st, stop=sp_)
            for half, pp in ((0, pa), (1, pb)):
                bg = bb + 8 * half
                nc.scalar.copy(y2[:, bg, 16 * hj:16 * hj + 16, :],
                               pp[:].rearrange("p (h w) -> p h w", h=16))
                nc.vector.bn_stats(stats6_2[:, 2 * bg + hj, :], pp[:])
        if bb % 2 == 1:
            pool2_chunk(slice(bb - 1, bb + 1))
            pool2_chunk(slice(8 + bb - 1, 8 + bb + 1))

    scale2, shift2 = _bn_reduce_apply(
        nc, statp, stats6_2, bng_s, d["cc2_in"], d["cc2_out"],
        n_shards=8, fold_halves=False, gcol=2, bcol=3, tg="2")

    p2 = p2p.tile([128, 16, 18, 18], BF)
    nc.gpsimd.memset(p2[:, :, 0:18:17, :], 0.0)
    nc.gpsimd.memset(p2[:, :, 1:17, 0:18:17], 0.0)
    nc.scalar.activation(p2[:, :, 1:17, 1:17], t2_p2[:], AF.Relu,
                         bias=shift2[:], scale=scale2[:])

    pclose(psb_cm, psa_cm)
    pclose(y2p_cm, pl2_cm)
    p3p_cm, p3p = popen("p3p", 1)
    w3p_cm, w3p = popen("w3p", 1)
    y3p_cm, y3p = popen("y3p", 1)

    # ------------------------------------------------------------- conv3

    psc_cm, psc = popen("psc", 4, "PSUM")
    w3t_s = w3p.tile([128, 2, 1, 9, 128], BF)
    for ct in range(2):
        nc.sync.dma_start(w3t_s[:, ct], d["w3t_d"][ct])
    y3 = y3p.tile([128, 2, 16, 18, 18], BF)
    nc.gpsimd.memset(y3[:, :, :, 0:18:17, :], 0.0)
    nc.gpsimd.memset(y3[:, :, :, 1:17, 0:18:17], 0.0)
    for ct in range(2):
        for bp in range(8):
            pst = psc.tile([128, 512], F32, tag="pc")
            for k, (dh, dw) in enumerate(OFF9):
                rhs = p2[:, 2 * bp:2 * bp + 2, dh:dh + 16, dw:dw + 16]
                nc.tensor.matmul(pst[:], w3t_s[:, ct, 0, k, :], rhs,
                                 start=(k == 0), stop=(k == 8))
            nc.scalar.activation(
                y3[:, ct, 2 * bp:2 * bp + 2, 1:17, 1:17],
                pst[:].rearrange("p (b h w) -> p b h w", b=2, h=16),
                AF.Relu, bias=cbs_s[:, ct:ct + 1])

    # --------------------------------------------------------- conv4

    w4p_cm, w4p = popen("w4p", 1)
    y4p_cm, y4p = popen("y4p", 1)
    pl3_cm, pl3 = popen("pl3", 2)
    w4t_s = w4p.tile([128, 2, 2, 9, 128], BF)
    for ct in range(2):
        nc.sync.dma_start(w4t_s[:, ct], d["w4t_d"][ct])
    y4r = y4p.tile([128, 2, 16, 18, 18], BF)
    nc.gpsimd.memset(y4r[:, :, :, 0:18:17, :], NEG)
    nc.gpsimd.memset(y4r[:, :, :, 1:17, 0:18:17], NEG)
    for ct in range(2):
        for bp in range(8):
            pst = psc.tile([128, 512], F32, tag="pc")
            first = True
            for kt in range(2):
                for k, (dh, dw) in enumerate(OFF9):
                    rhs = y3[:, kt, 2 * bp:2 * bp + 2, dh:dh + 16, dw:dw + 16]
                    nc.tensor.matmul(pst[:], w4t_s[:, ct, kt, k, :], rhs,
                                     start=first, stop=(kt == 1 and k == 8))
                    first = False
            nc.scalar.copy(
                y4r[:, ct, 2 * bp:2 * bp + 2, 1:17, 1:17],
                pst[:].rearrange("p (b h w) -> p b h w", b=2, h=16))

    # pool3 + bias+relu -> p3 [128, 2, 16, 10, 10]
    p3 = p3p.tile([128, 2, 16, 10, 10], BF)
    nc.gpsimd.memset(p3[:, :, :, 0:10:9, :], 0.0)
    nc.gpsimd.memset(p3[:, :, :, 1:9, 0:10:9], 0.0)
    for ct in range(2):
        t1 = pl3.tile([128, 16, 18, 8], BF, tag="t1c")
        nc.vector.tensor_tensor(t1[:], y4r[:, ct, :, :, 0:16:2],
                                y4r[:, ct, :, :, 1:17:2], ALU.max)
        nc.vector.tensor_tensor(t1[:], t1[:], y4r[:, ct, :, :, 2:18:2], ALU.max)
        t2 = pl3.tile([128, 16, 8, 8], BF, tag="t2c")
        nc.vector.tensor_tensor(t2[:], t1[:, :, 0:16:2, :], t1[:, :, 1:17:2, :], ALU.max)
        nc.vector.tensor_tensor(t2[:], t2[:], t1[:, :, 2:18:2, :], ALU.max)
        nc.scalar.activation(p3[:, ct, :, 1:9, 1:9], t2[:], AF.Relu,
                             bias=cbs_s[:, 2 + ct:3 + ct])

    pclose(pl3_cm, y4p_cm, w4p_cm, y3p_cm, w3p_cm)
    p4p_cm, p4p = popen("p4p", 1)
    y7p_cm, y7p_pool = popen("y7p", 1)
    cwA_cm, cwA = popen("cwA", 1)
    w1fA = cwA.tile([128, 8, 4, 5, 128], BF, tag="w1fA")
    cwB_cm, cwB = popen("cwB", 1)
    w1fB = cwB.tile([128, 8, 4, 5, 128], BF, tag="w1fB")
    y5p_cm, y5p = popen("y5p", 1)
    wstg_cm, wstg = popen("wstg", 2)
    pl4_cm, pl4 = popen("pl4", 2)

    # ----------------------------------------------------- conv5

    y5 = y5p.tile([128, 4, 16, 10, 10], BF, tag="y5")
    nc.gpsimd.memset(y5[:, :, :, 0:10:9, :], 0.0)
    nc.gpsimd.memset(y5[:, :, :, 1:9, 0:10:9], 0.0)
    for ct in range(4):
        w5c = wstg.tile([128, 2, 9, 128], BF, tag="wc5", name=f"w5c{ct}")
        nc.sync.dma_start(w5c[:], d["w5t_d"][ct])
        for bp in range(2):
            pst = psc.tile([128, 512], F32, tag="pc")
            first = True
            for kt in range(2):
                for k, (dh, dw) in enumerate(OFF9):
                    rhs = p3[:, kt, 8 * bp:8 * bp + 8, dh:dh + 8, dw:dw + 8]
                    nc.tensor.matmul(pst[:], w5c[:, kt, k, :], rhs,
                                     start=first, stop=(kt == 1 and k == 8))
                    first = False
            nc.scalar.activation(
                y5[:, ct, 8 * bp:8 * bp + 8, 1:9, 1:9],
                pst[:].rearrange("p (b h w) -> p b h w", b=8, h=8),
                AF.Relu, bias=cbs_s[:, 4 + ct:5 + ct])

    # ------------------------------------------------- conv6

    y6r = y5p.tile([128, 4, 16, 10, 10], BF, tag="y6r")
    nc.gpsimd.memset(y6r[:, :, :, 0:10:9, :], NEG)
    nc.gpsimd.memset(y6r[:, :, :, 1:9, 0:10:9], NEG)
    for ct in range(4):
        w6c = wstg.tile([128, 4, 9, 128], BF, tag="wc6", name=f"w6c{ct}")
        nc.sync.dma_start(w6c[:], d["w6t_d"][ct])
        nc.sync.dma_start(w1fA[:, 2 * ct:2 * ct + 2], d["w1f_d"][:, 2 * ct:2 * ct + 2])
        for bp in range(2):
            pst = psc.tile([128, 512], F32, tag="pc")
            first = True
            for kt in range(4):
                for k, (dh, dw) in enumerate(OFF9):
                    rhs = y5[:, kt, 8 * bp:8 * bp + 8, dh:dh + 8, dw:dw + 8]
                    nc.tensor.matmul(pst[:], w6c[:, kt, k, :], rhs,
                                     start=first, stop=(kt == 3 and k == 8))
                    first = False
            nc.scalar.copy(
                y6r[:, ct, 8 * bp:8 * bp + 8, 1:9, 1:9],
                pst[:].rearrange("p (b h w) -> p b h w", b=8, h=8))
    # pool4 + bias+relu -> p4 [128, 4, 16, 6, 6]
    p4 = p4p.tile([128, 4, 16, 6, 6], BF)
    nc.gpsimd.memset(p4[:, :, :, 0:6:5, :], 0.0)
    nc.gpsimd.memset(p4[:, :, :, 1:5, 0:6:5], 0.0)
    for ct in range(4):
        t1 = pl4.tile([128, 16, 10, 4], BF, tag="t1d")
        nc.vector.tensor_tensor(t1[:], y6r[:, ct, :, :, 0:8:2],
                                y6r[:, ct, :, :, 1:9:2], ALU.max)
        nc.vector.tensor_tensor(t1[:], t1[:], y6r[:, ct, :, :, 2:10:2], ALU.max)
        t2 = pl4.tile([128, 16, 4, 4], BF, tag="t2d")
        nc.vector.tensor_tensor(t2[:], t1[:, :, 0:8:2, :], t1[:, :, 1:9:2, :], ALU.max)
        nc.vector.tensor_tensor(t2[:], t2[:], t1[:, :, 2:10:2, :], ALU.max)
        nc.scalar.activation(p4[:, ct, :, 1:5, 1:5], t2[:], AF.Relu,
                             bias=cbs_s[:, 8 + ct:9 + ct])

    pclose(pl4_cm)


    # ------------------------------------------------- conv7
    y7 = y7p_pool.tile([128, 4, 16, 16], BF, name="y7")
    for ct in range(4):
        w7c = wstg.tile([128, 4, 9, 128], BF, tag="wc6", name=f"w7c{ct}")
        nc.sync.dma_start(w7c[:], d["w7t_d"][ct])
        nc.sync.dma_start(w1fB[:, 2 * ct:2 * ct + 2], d["w1f_d"][:, 8 + 2 * ct:10 + 2 * ct])
        pst = psc.tile([128, 256], F32, tag="pc7")
        first = True
        for kt in range(4):
            for k, (dh, dw) in enumerate(OFF9):
                rhs = p4[:, kt, :, dh:dh + 4, dw:dw + 4]
                nc.tensor.matmul(pst[:], w7c[:, kt, k, :], rhs,
                                 start=first, stop=(kt == 3 and k == 8))
                first = False
        nc.scalar.activation(
            y7[:, ct, :, :],
            pst[:].rearrange("p (b n) -> p b n", b=16),
            AF.Relu, bias=cbs_s[:, 12 + ct:13 + ct])

    pclose(psc_cm)
    pclose(wstg_cm, y5p_cm)

    # --------------------------------------------- colons

    colp_cm, colp = popen("colp", 1)
    hp_cm, hp = popen("hp", 3)
    sxp_cm, sxp = popen("sxp", 4)
    colw_cm, colw = popen("colw", 1)
    psU_cm, psU = popen("psU", 3, "PSUM")
    psL_cm, psL = popen("psL", 3, "PSUM")
    psT_cm, psT = popen("psT", 1, "PSUM")
    wnp_cm, wnp = popen("wnp", 3)
    b1r_s = colw.tile([1, 16, 5, 128], BF)
    nc.sync.dma_start(b1r_s[:], d["b1r_d"][:])
    w2c_s = colw.tile([128, 16, 5, 10], BF)
    nc.sync.dma_start(w2c_s[:], d["w2c_d"][:])
    b2r_s = colw.tile([1, 16, 10], BF)
    nc.sync.dma_start(b2r_s[:], d["b2r_d"][:])
    ones1 = colw.tile([1, 16], BF)
    nc.gpsimd.memset(ones1[:], 1.0)

    U_sb = colp.tile([128, 16, 5, 16], F32)
    h1 = colp.tile([128, 16, 5, 16], BF)
    preds1 = colp.tile([16, 16, 10], F32)
    preds2 = colp.tile([16, 16, 10], F32)

    def colon_tail(n, h_t, logits_buf):
        L_ps = psL.tile([16, 10], F32, tag="lp")
        for kt in range(5):
            nc.tensor.matmul(L_ps[:], h_t[:, kt, :], w2c_s[:, n, kt, :],
                             start=(kt == 0), stop=False)
        nc.tensor.matmul(L_ps[:], ones1[:], b2r_s[:, n, :], start=False, stop=True)
        nc.scalar.copy(logits_buf[:, n, :], L_ps[:])

    def batched_softmax(logits_buf, preds_out):
        # softmax over the last dim for all 16 nodes in 6 ops
        mx = sxp.tile([16, 16], F32, tag="mx")
        nc.vector.tensor_reduce(mx[:], logits_buf[:], AX.X, ALU.max, negate=True)
        sh = sxp.tile([16, 16, 10], F32, tag="sh")
        nc.vector.tensor_tensor(sh[:], logits_buf[:],
                                mx[:, :, None].to_broadcast((16, 16, 10)), ALU.add)
        ex = sxp.tile([16, 16, 10], F32, tag="ex")
        nc.scalar.activation(ex[:], sh[:], AF.Exp)
        sm = sxp.tile([16, 16], F32, tag="sm")
        nc.vector.tensor_reduce(sm[:], ex[:], AX.X, ALU.add)
        rc = sxp.tile([16, 16], F32, tag="rc")
        nc.vector.reciprocal(rc[:], sm[:])
        nc.vector.tensor_tensor(preds_out[:], ex[:],
                                rc[:, :, None].to_broadcast((16, 16, 10)), ALU.mult)

    # pass 1
    logits1 = colp.tile([16, 16, 10], F32)
    logits2 = colp.tile([16, 16, 10], F32)
    for n in range(16):
        w1n = (w1fA if n < 8 else w1fB)[:, n % 8]
        up = psU.tile([128, 5, 16], F32, tag="up")
        for mt in range(5):
            for kt in range(4):
                nc.tensor.matmul(up[:, mt, :], w1n[:, kt, mt, :], y7[:, kt, :, n],
                                 start=(kt == 0), stop=False)
            nc.tensor.matmul(up[:, mt, :], b1r_s[:, n, mt, :], ones1[:],
                             start=False, stop=(mt == 4))
        nc.vector.tensor_copy(U_sb[:, n, :, :], up[:])
        nc.scalar.activation(h1[:, n, :, :], up[:], AF.Tanh)
        colon_tail(n, h1[:, n, :, :], logits1)
    batched_softmax(logits1, preds1)

    # transpose preds1 -> predsT [128|32, 16]
    pv = preds1[:].rearrange("p n c -> p (n c)")
    predsT = colp.tile([128, 2, 16], BF)
    nc.gpsimd.memset(predsT[:, 1, :], 0.0)
    ptp = psT.tile([128, 16], F32, tag="ptp")
    nc.tensor.transpose(ptp[:], pv[:, 0:128], ident[0:16, 0:16])
    nc.scalar.copy(predsT[:, 0, :], ptp[:])
    ptp2 = psT.tile([32, 16], F32, tag="ptp2")
    nc.tensor.transpose(ptp2[:], pv[:, 128:160], ident[0:16, 0:16])
    nc.scalar.copy(predsT[0:32, 1, :], ptp2[:])

    # pass 2
    for n in range(16):
        wtn = wnp.tile([128, 2, 5, 128], BF, tag="wtn", name=f"wtn{n}")
        nc.sync.dma_start(wtn[:], d["wt_d"][:, n])
        h2 = hp.tile([128, 5, 16], BF, tag="h2")
        dp = psU.tile([128, 5, 16], F32, tag="up")
        for mt in range(5):
            for kt in range(2):
                nc.tensor.matmul(dp[:, mt, :], wtn[:, kt, mt, :], predsT[:, kt, :],
                                 start=(kt == 0), stop=(kt == 1 and mt == 4))
        vv = sxp.tile([128, 5, 16], F32, tag="vv")
        nc.vector.tensor_tensor(vv[:], U_sb[:, n, :, :], dp[:], ALU.add)
        nc.scalar.activation(h2[:], vv[:], AF.Tanh)
        colon_tail(n, h2[:], logits2)
    batched_softmax(logits2, preds2)

    nc.sync.dma_start(d["out_d"][:], preds2[:])

    pclose(psT_cm, psL_cm, psU_cm)
    pclose(wnp_cm, colw_cm, sxp_cm, hp_cm, colp_cm, cwB_cm, cwA_cm, y7p_cm,
           p4p_cm, p3p_cm, p2p_cm, p1p_cm, statp_cm, const_cm)


_NC_CACHE = None


def kernel(**inputs):
    global _NC_CACHE, _LAST_RESULT
    g = _prep(inputs)
    xi_full = g.pop("_xi_full")

    if _NC_CACHE is None:
        _NC_CACHE = _build()
    nc = _NC_CACHE

    in_maps = []
    for c in range(N_CORES):
        m = {"xi": np.ascontiguousarray(xi_full[:, c * BL:(c + 1) * BL])}
        m.update(g)
        in_maps.append(m)

    res = bass_utils.run_bass_kernel_spmd(nc, in_maps, core_ids=list(range(N_CORES)))
    _LAST_RESULT = res

    preds_all = np.concatenate([res.results[c]["preds2"] for c in range(N_CORES)], axis=0)
    preds2 = np.ascontiguousarray(preds_all.transpose(1, 0, 2)).astype(np.float32)
    sum_preds = preds2.mean(axis=0)
    return (sum_preds * sum_preds).astype(np.float32), preds2


# revision 31
# speedup vs baseline: 1.1073x; 1.0955x over previous
"""Trainium2 Bass kernel for nn_Brain (7-conv CNN backbone + BN(train) +
16 per-node MLP colons with one message-passing round), data-parallel over
batch across 8 NeuronCores.

Self-contained: hardcodes all shapes/sharding. Returns (squared_sum_preds,
preds2) like the reference.
"""
import numpy as np
import ml_dtypes

import concourse.bass as bass
import concourse.mybir as mybir
import concourse.tile as tile
from concourse import bass_utils
from concourse.masks import make_identity
from concourse.vector_clock import ScopedClock

bf16 = ml_dtypes.bfloat16
F32 = mybir.dt.float32
BF = mybir.dt.bfloat16
AF = mybir.ActivationFunctionType
ALU = mybir.AluOpType
AX = mybir.AxisListType

N_CORES = 8
B = 128
BL = 16            # batch per core
NEG = -1.0e30      # -inf-ish padding for raw max-pools
EPS = 1e-5
OFF9 = [(dh, dw) for dh in range(3) for dw in range(3)]
RG = [list(range(N_CORES))]

_LAST_RESULT = None


# ---------------------------------------------------------------------------
# compat TileContext: this container's walrus accepts at most ONE sync wait
# per instruction; split extra waits onto NOPs on the same engine.
_REAL_ENGINES = {
    mybir.EngineType.PE,
    mybir.EngineType.DVE,
    mybir.EngineType.Activation,
    mybir.EngineType.Pool,
    mybir.EngineType.SP,
}


class CompatTileContext(tile.TileContext):
    MAX_SYNC = 1

    def _commit_instruction(self, inst, lazy_reg_writes=True):
        si = getattr(inst, "sync_info", None)
        if (
            si is not None
            and si.on_wait
            and len(si.on_wait) > 1
            and inst.engine in _REAL_ENGINES
        ):
            waits = list(si.on_wait)
            si.on_wait = waits[-1:]
            for k, w in enumerate(waits[:-1]):
                nop = mybir.InstNoOp(
                    name=f"{inst.name}-xw{k}",
                    sync_info=mybir.SyncInfo(on_wait=[w], on_update=[]),
                    bass_nofuse=True,
                    engine=inst.engine,
                )
                super()._commit_instruction(nop, lazy_reg_writes=False)
        return super()._commit_instruction(inst, lazy_reg_writes)

    def _drain_and_barrier(self, tick_clock, wait_clock):
        nop0 = self.nc.sync.nop(nofuse=True)
        wait_clock.add_sem_waits(nop0.ins, ScopedClock({None: tick_clock.global_clock}))
        si = nop0.ins.sync_info
        waits = list(si.on_wait) if si is not None and si.on_wait else []
        if len(waits) > self.MAX_SYNC:
            si.on_wait = waits[: self.MAX_SYNC]
            rest = waits[self.MAX_SYNC:]
            while rest:
                nop = self.nc.sync.nop(nofuse=True)
                nsi = nop.ins.sync_info
                chunk, rest = rest[: self.MAX_SYNC], rest[self.MAX_SYNC:]
                if nsi is None:
                    nop.ins.sync_info = mybir.SyncInfo(on_wait=chunk, on_update=[])
                else:
                    nsi.on_wait = chunk
        self.nc.sync.drain()

        self.nc.all_engine_barrier()
        assert self.sems is not None
        popped = self.nc._tile_sem_poison_stack.pop()
        assert popped is self._sem_poison
        self.nc.clear_and_free_semaphores(list(self.sems.allocated().values()))
        self.nc.all_engine_barrier()


# ---------------------------------------------------------------------------
# host-side weight preparation

def _neighbors(i, w, h):
    size = w * h
    out = []
    if i - w >= 0:
        out.append(i - w)
    if i % w != 0:
        out.append(i - 1)
    if (i + 1) % w != 0:
        out.append(i + 1)
    if i + w < size:
        out.append(i + w)
    if i - w - 1 >= 0 and i % w != 0:
        out.append(i - w - 1)
    if i - w + 1 >= 0 and (i + 1) % w != 0:
        out.append(i - w + 1)
    if i + w - 1 < size and i % w != 0:
        out.append(i + w - 1)
    if i + w + 1 < size and (i + 1) % w != 0:
        out.append(i + w + 1)
    return out


def _prep(inputs):
    f = np.float32
    g = {}

    cw1 = np.asarray(inputs["cw1"], f)
    g["w1t"] = np.ascontiguousarray(cw1.transpose(2, 3, 1, 0).reshape(27, 64)).astype(bf16)

    x = np.asarray(inputs["x"], f).astype(bf16)        # [128, 3, 64, 64]
    xt = x.transpose(1, 0, 2, 3)                        # [3, 128, 64, 64]
    xi = np.zeros((27, 128, 64, 64), bf16)
    for k, (dh, dw) in enumerate([(a, b) for a in range(3) for b in range(3)]):
        hs, he = max(0, 1 - dh), 64 + min(0, 1 - dh)
        ws, we = max(0, 1 - dw), 64 + min(0, 1 - dw)
        xi[3 * k:3 * k + 3, :, hs:he, ws:we] = \
            xt[:, :, hs + dh - 1:he + dh - 1, ws + dw - 1:we + dw - 1]
    g["_xi_full"] = xi

    w2 = np.asarray(inputs["cw2"], f).transpose(2, 3, 1, 0).reshape(9, 64, 128)
    g["w2t"] = np.ascontiguousarray(
        np.concatenate([w2, w2], axis=1).transpose(1, 0, 2)).astype(bf16)  # [128,9,128]

    def conv_t(cw, kt, ct):
        a = np.asarray(cw, f).transpose(2, 3, 1, 0).reshape(9, kt, 128, ct, 128)
        return np.ascontiguousarray(a.transpose(3, 2, 1, 0, 4)).astype(bf16)  # [ct,p,kt,k,q]

    g["w3t"] = conv_t(inputs["cw3"], 1, 2)
    g["w4t"] = conv_t(inputs["cw4"], 2, 2)
    g["w5t"] = conv_t(inputs["cw5"], 2, 4)
    g["w6t"] = conv_t(inputs["cw6"], 4, 4)
    g["w7t"] = conv_t(inputs["cw7"], 4, 4)

    cbs = np.zeros((128, 16), f)
    cbs[:, 0:2] = np.asarray(inputs["cb3"], f).reshape(2, 128).T
    cbs[:, 2:4] = np.asarray(inputs["cb4"], f).reshape(2, 128).T
    cbs[:, 4:8] = np.asarray(inputs["cb5"], f).reshape(4, 128).T
    cbs[:, 8:12] = np.asarray(inputs["cb6"], f).reshape(4, 128).T
    cbs[:, 12:16] = np.asarray(inputs["cb7"], f).reshape(4, 128).T
    g["cbs"] = cbs

    bng = np.zeros((128, 4), f)
    bng[:, 0] = np.tile(np.asarray(inputs["bn1_g"], f), 2)
    bng[:, 1] = np.tile(np.asarray(inputs["bn1_b"], f), 2)
    bng[:, 2] = np.asarray(inputs["bn2_g"], f)
    bng[:, 3] = np.asarray(inputs["bn2_b"], f)
    g["bng"] = bng

    W1 = np.asarray(inputs["W1"], f)        # [16, 592, 600]
    b1 = np.asarray(inputs["b1"], f)        # [16, 600]
    W2 = np.asarray(inputs["W2"], f)        # [16, 600, 10]
    b2 = np.asarray(inputs["b2"], f)        # [16, 10]

    w1f = np.zeros((16, 512, 640), f)
    w1f[:, :, :600] = W1[:, 80:, :]
    g["w1f"] = np.ascontiguousarray(
        w1f.reshape(16, 4, 128, 5, 128).transpose(2, 0, 1, 3, 4)).astype(bf16)

    idxs = np.zeros((16, 8), np.int64)
    mask = np.zeros((16, 8), f)
    for i in range(16):
        nb = _neighbors(i, 4, 4)
        idxs[i, :len(nb)] = nb
        mask[i, :len(nb)] = 1.0
    wt = np.zeros((16, 256, 640), f)
    for n in range(16):
        G = np.zeros((80, 160), f)
        for j in range(8):
            if mask[n, j] > 0:
                for c in range(10):
                    G[10 * j + c, 10 * idxs[n, j] + c] = 1.0
        wt[n, :160, :600] = G.T @ W1[n, :80, :]
    g["wt"] = np.ascontiguousarray(
        wt.reshape(16, 2, 128, 5, 128).transpose(2, 0, 1, 3, 4)).astype(bf16)
    g["b2r"] = np.ascontiguousarray(b2.reshape(1, 16, 10)).astype(bf16)

    b1p = np.zeros((16, 640), f)
    b1p[:, :600] = b1
    g["b1r"] = np.ascontiguousarray(b1p.reshape(1, 16, 5, 128)).astype(bf16)

    w2p = np.zeros((16, 640, 10), f)
    w2p[:, :600] = W2
    g["w2c"] = np.ascontiguousarray(
        w2p.reshape(16, 5, 128, 10).transpose(2, 0, 1, 3)).astype(bf16)
    return g


# ---------------------------------------------------------------------------
# device program

def _build():
    nc = bass.Bass()
    d = {}
    d["x_d"] = nc.dram_tensor("xi", [27, BL, 64, 64], BF, kind="ExternalInput")
    d["w1t_d"] = nc.dram_tensor("w1t", [27, 64], BF, kind="ExternalInput")
    d["w2t_d"] = nc.dram_tensor("w2t", [128, 9, 128], BF, kind="ExternalInput")
    d["w3t_d"] = nc.dram_tensor("w3t", [2, 128, 1, 9, 128], BF, kind="ExternalInput")
    d["w4t_d"] = nc.dram_tensor("w4t", [2, 128, 2, 9, 128], BF, kind="ExternalInput")
    d["w5t_d"] = nc.dram_tensor("w5t", [4, 128, 2, 9, 128], BF, kind="ExternalInput")
    d["w6t_d"] = nc.dram_tensor("w6t", [4, 128, 4, 9, 128], BF, kind="ExternalInput")
    d["w7t_d"] = nc.dram_tensor("w7t", [4, 128, 4, 9, 128], BF, kind="ExternalInput")
    d["cbs_d"] = nc.dram_tensor("cbs", [128, 16], F32, kind="ExternalInput")
    d["bng_d"] = nc.dram_tensor("bng", [128, 4], F32, kind="ExternalInput")
    d["w1f_d"] = nc.dram_tensor("w1f", [128, 16, 4, 5, 128], BF, kind="ExternalInput")
    d["wt_d"] = nc.dram_tensor("wt", [128, 16, 2, 5, 128], BF, kind="ExternalInput")
    d["b1r_d"] = nc.dram_tensor("b1r", [1, 16, 5, 128], BF, kind="ExternalInput")
    d["w2c_d"] = nc.dram_tensor("w2c", [128, 16, 5, 10], BF, kind="ExternalInput")
    d["b2r_d"] = nc.dram_tensor("b2r", [1, 16, 10], BF, kind="ExternalInput")
    d["out_d"] = nc.dram_tensor("preds2", [BL, 16, 10], F32, kind="ExternalOutput")

    d["cc1_in"] = nc.dram_tensor("cc1_in", [128, 2], F32)
    d["cc1_out"] = nc.dram_tensor("cc1_out", [128, 2], F32, addr_space="Shared")
    d["cc2_in"] = nc.dram_tensor("cc2_in", [128, 2], F32)
    d["cc2_out"] = nc.dram_tensor("cc2_out", [128, 2], F32, addr_space="Shared")

    with CompatTileContext(nc, pool_alloc_mode="queue") as tc:
        _trace(nc, tc, d)
    return nc


def _bn_reduce_apply(nc, sp, stats6, bng_s, cc_in, cc_out, n_shards, fold_halves,
                     gcol, bcol, tg):
    """bn_aggr -> AllReduce -> (optional half fold) -> scale/shift [128,1]."""
    st2 = sp.tile([128, 2], F32, tag="st2" + tg)
    nc.vector.bn_aggr(st2[:], stats6[:])
    pay = sp.tile([128, 2], F32, tag="pay" + tg)
    nc.vector.tensor_copy(pay[:, 0:1], st2[:, 0:1])
    nc.vector.tensor_tensor(pay[:, 1:2], st2[:, 0:1], st2[:, 0:1], ALU.mult)
    nc.vector.tensor_tensor(pay[:, 1:2], pay[:, 1:2], st2[:, 1:2], ALU.add)
    nc.sync.dma_start(cc_in[:], pay[:])
    nc.gpsimd.collective_compute(
        "AllReduce", ALU.add, ins=[cc_in[:]], outs=[cc_out[:]], replica_groups=RG)
    tot = sp.tile([128, 2], F32, tag="tot" + tg)
    nc.sync.dma_start(tot[:], cc_out[:])
    if fold_halves:
        sw = sp.tile([128, 2], F32, tag="sw" + tg)
        nc.sync.dma_start(sw[0:64, :], cc_out[64:128, :])
        nc.sync.dma_start(sw[64:128, :], cc_out[0:64, :])
        nc.vector.tensor_tensor(tot[:], tot[:], sw[:], ALU.add)
    gm = sp.tile([128, 1], F32, tag="gm" + tg)
    nc.vector.tensor_scalar_mul(gm[:], tot[:, 0:1], 1.0 / n_shards)
    gv = sp.tile([128, 1], F32, tag="gv" + tg)
    nc.vector.tensor_scalar_mul(gv[:], tot[:, 1:2], 1.0 / n_shards)
    gm2 = sp.tile([128, 1], F32, tag="gm2" + tg)
    nc.vector.tensor_tensor(gm2[:], gm[:], gm[:], ALU.mult)
    nc.vector.tensor_tensor(gv[:], gv[:], gm2[:], ALU.subtract)
    epst = sp.tile([128, 1], F32, tag="eps" + tg)
    nc.gpsimd.memset(epst[:], EPS)
    sd = sp.tile([128, 1], F32, tag="sd" + tg)
    nc.scalar.activation(sd[:], gv[:], AF.Sqrt, bias=epst[:])
    inv = sp.tile([128, 1], F32, tag="inv" + tg)
    nc.vector.reciprocal(inv[:], sd[:])
    scale = sp.tile([128, 1], F32, tag="scale" + tg)
    nc.vector.tensor_tensor(scale[:], inv[:], bng_s[:, gcol:gcol + 1], ALU.mult)
    tmp = sp.tile([128, 1], F32, tag="tmp" + tg)
    nc.vector.tensor_tensor(tmp[:], gm[:], scale[:], ALU.mult)
    shift = sp.tile([128, 1], F32, tag="shift" + tg)
    nc.vector.tensor_tensor(shift[:], bng_s[:, bcol:bcol + 1], tmp[:], ALU.subtract)
    return scale, shift


def _trace(nc, tc, d):
    x_d = d["x_d"]

    def popen(name, bufs, space="SBUF"):
        p = tc.alloc_tile_pool(name=name, bufs=bufs, space=space)
        return p, p

    def pclose(*pools):
        for p in pools:
            p.release()

    # Pool discipline: release is strict LIFO. "Permanent" pools (small or
    # suffix-lived) open just-in-time and close only at the very end; big
    # phase temporaries live in nested scopes.
    const_cm, const = popen("const", 1)
    statp_cm, statp = popen("stats", 1)
    p1p_cm, p1p = popen("p1p", 1)
    icp_cm, icp = popen("icp", 1)
    y1p_cm, y1p = popen("y1p", 1)
    pl1_cm, pl1 = popen("pl1", 1)
    w1t_s = const.tile([27, 64], BF)
    nc.sync.dma_start(w1t_s[:], d["w1t_d"][:])
    w2t_s = const.tile([128, 9, 128], BF)
    nc.sync.dma_start(w2t_s[:], d["w2t_d"][:])
    cbs_s = const.tile([128, 16], F32)
    nc.sync.dma_start(cbs_s[:], d["cbs_d"][:])
    bng_s = const.tile([128, 4], F32)
    nc.sync.dma_start(bng_s[:], d["bng_d"][:])
    ident = const.tile([128, 128], F32)
    make_identity(nc, ident[:])

    # ------------------------------------------------------------- conv1

    ps1_cm, ps1 = popen("ps1", 6, "PSUM")
    y1 = y1p.tile([128, 8, 64, 64], BF)
    ics = [icp.tile([27, 4, 64, 64], BF, tag=f"ic{i}", name=f"ic{i}") for i in range(2)]

    stats6_1 = statp.tile([128, 64, 6], F32, tag="s61")
    t2_p1 = pl1.tile([128, 8, 32, 32], BF, tag="t2")

    for r in range(4):
        ic = ics[r % 2]
        nc.sync.dma_start(ic[0:27, 0:2], x_d[:, 2 * r:2 * r + 2])
        nc.sync.dma_start(ic[0:27, 2:4], x_d[:, 8 + 2 * r:8 + 2 * r + 2])
        for bs in range(2):
            for hj in range(8):
                pst = ps1.tile([128, 512], F32, tag="pa")
                rl = ic[0:27, bs, 8 * hj:8 * hj + 8, :]
                ru = ic[0:27, 2 + bs, 8 * hj:8 * hj + 8, :]
                nc.tensor.matmul(pst[0:64, :], w1t_s[:], rl, start=True, stop=True)
                nc.tensor.matmul(pst[64:128, :], w1t_s[:], ru, start=True, stop=True,
                                 tile_position=(0, 64))
                nc.scalar.copy(y1[:, 2 * r + bs, 8 * hj:8 * hj + 8, :],
                               pst[:].rearrange("p (h w) -> p h w", h=8))
                ci = 16 * r + 8 * bs + hj
                nc.vector.bn_stats(stats6_1[:, ci, :], pst[:])
        if True:
            b4 = 2 * r
            t1 = pl1.tile([128, 2, 64, 32], BF, tag="t1", name=f"t1_{b4}")
            ys = y1[:, b4:b4 + 2]
            nc.vector.tensor_tensor(t1[:, :, :, 1:32], ys[:, :, :, 1:63:2], ys[:, :, :, 2:64:2], ALU.max)
            nc.vector.tensor_tensor(t1[:, :, :, 1:32], t1[:, :, :, 1:32], ys[:, :, :, 3:64:2], ALU.max)
            nc.vector.tensor_tensor(t1[:, :, :, 0:1], ys[:, :, :, 0:1], ys[:, :, :, 1:2], ALU.max)
            ts = t2_p1[:, b4:b4 + 2]
            nc.vector.tensor_tensor(ts[:, :, 1:32, :], t1[:, :, 1:63:2, :], t1[:, :, 2:64:2, :], ALU.max)
            nc.vector.tensor_tensor(ts[:, :, 1:32, :], ts[:, :, 1:32, :], t1[:, :, 3:64:2, :], ALU.max)
            nc.vector.tensor_tensor(ts[:, :, 0:1, :], t1[:, :, 0:1, :], t1[:, :, 1:2, :], ALU.max)

    scale1, shift1 = _bn_reduce_apply(
        nc, statp, stats6_1, bng_s, d["cc1_in"], d["cc1_out"],
        n_shards=16, fold_halves=True, gcol=0, bcol=1, tg="1")

    # pool1 (raw, per batch) then affine+relu into p1
    p1 = p1p.tile([128, 8, 34, 34], BF)
    nc.gpsimd.memset(p1[:, :, 0:34:33, :], 0.0)
    nc.gpsimd.memset(p1[:, :, 1:33, 0:34:33], 0.0)
    for b4 in range(4):
        nc.scalar.activation(p1[:, 2 * b4:2 * b4 + 2, 1:33, 1:33],
                             t2_p1[:, 2 * b4:2 * b4 + 2], AF.Relu,
                             bias=shift1[:], scale=scale1[:])

    pclose(ps1_cm)
    pclose(pl1_cm, y1p_cm, icp_cm)
    p2p_cm, p2p = popen("p2p", 1)
    pl2_cm, pl2 = popen("pl2", 1)
    y2p_cm, y2p = popen("y2p", 1)

    # ------------------------------------------------------------- conv2

    psa_cm, ps2a = popen("ps2a", 4, "PSUM")
    psb_cm, ps2b = popen("ps2b", 4, "PSUM")
    y2 = y2p.tile([128, 16, 32, 32], BF)
    stats6_2 = statp.tile([128, 32, 6], F32, tag="s62")
    t2_p2 = pl2.tile([128, 16, 16, 16], BF, tag="t2b")

    def pool2_chunk(bsl):
        t1 = pl2.tile([128, 2, 32, 16], BF, tag="t1b", name=f"t1b{bsl.start}")
        ys = y2[:, bsl]
        nc.vector.tensor_tensor(t1[:, :, :, 1:16], ys[:, :, :, 1:31:2], ys[:, :, :, 2:32:2], ALU.max)
        nc.vector.tensor_tensor(t1[:, :, :, 1:16], t1[:, :, :, 1:16], ys[:, :, :, 3:32:2], ALU.max)
        nc.vector.tensor_tensor(t1[:, :, :, 0:1], ys[:, :, :, 0:1], ys[:, :, :, 1:2], ALU.max)
        ts = t2_p2[:, bsl]
        nc.vector.tensor_tensor(ts[:, :, 1:16, :], t1[:, :, 1:31:2, :], t1[:, :, 2:32:2, :], ALU.max)
        nc.vector.tensor_tensor(ts[:, :, 1:16, :], ts[:, :, 1:16, :], t1[:, :, 3:32:2, :], ALU.max)
        nc.vector.tensor_tensor(ts[:, :, 0:1, :], t1[:, :, 0:1, :], t1[:, :, 1:2, :], ALU.max)

    for bb in range(8):
        for hj in range(2):
            pa = ps2a.tile([128, 512], F32, tag="pa2")
            pb = ps2b.tile([128, 512], F32, tag="pb2")
            for k, (dh, dw) in enumerate(OFF9):
                st, sp_ = (k == 0), (k == 8)
                rl = p1[0:64, bb, 16 * hj + dh:16 * hj + dh + 16, dw:dw + 32]
                ru = p1[64:128, bb, 16 * hj + dh:16 * hj + dh + 16, dw:dw + 32]
                nc.tensor.matmul(pa[:], w2t_s[0:64, k, :], rl, start=st, stop=sp_)
                nc.tensor.matmul(pb[:], w2t_s[64:128, k, :], ru, start=st, stop=sp_)
            for half, pp in ((0, pa), (1, pb)):
                bg = bb + 8 * half
                nc.scalar.copy(y2[:, bg, 16 * hj:16 * hj + 16, :],
                               pp[:].rearrange("p (h w) -> p h w", h=16))
                nc.vector.bn_stats(stats6_2[:, 2 * bg + hj, :], pp[:])
        if bb % 2 == 1:
            pool2_chunk(slice(bb - 1, bb + 1))
            pool2_chunk(slice(8 + bb - 1, 8 + bb + 1))

    scale2, shift2 = _bn_reduce_apply(
        nc, statp, stats6_2, bng_s, d["cc2_in"], d["cc2_out"],
        n_shards=8, fold_halves=False, gcol=2, bcol=3, tg="2")

    p2 = p2p.tile([128, 16, 18, 18], BF)
    nc.gpsimd.memset(p2[:, :, 0:18:17, :], 0.0)
    nc.gpsimd.memset(p2[:, :, 1:17, 0:18:17], 0.0)
    for b4 in range(4):
        nc.scalar.activation(p2[:, 4 * b4:4 * b4 + 4, 1:17, 1:17],
                             t2_p2[:, 4 * b4:4 * b4 + 4], AF.Relu,
                             bias=shift2[:], scale=scale2[:])

    pclose(psb_cm, psa_cm)
    pclose(y2p_cm, pl2_cm)
    p3p_cm, p3p = popen("p3p", 1)
    w3p_cm, w3p = popen("w3p", 1)
    y3p_cm, y3p = popen("y3p", 1)

    # ------------------------------------------------------------- conv3

    psc_cm, psc = popen("psc", 4, "PSUM")
    w3t_s = w3p.tile([128, 2, 1, 9, 128], BF)
    for ct in range(2):
        nc.sync.dma_start(w3t_s[:, ct], d["w3t_d"][ct])
    y3 = y3p.tile([128, 2, 16, 18, 18], BF)
    nc.gpsimd.memset(y3[:, :, :, 0:18:17, :], 0.0)
    nc.gpsimd.memset(y3[:, :, :, 1:17, 0:18:17], 0.0)
    for ct in range(2):
        for bp in range(8):
            pst = psc.tile([128, 512], F32, tag="pc")
            for k, (dh, dw) in enumerate(OFF9):
                rhs = p2[:, 2 * bp:2 * bp + 2, dh:dh + 16, dw:dw + 16]
                nc.tensor.matmul(pst[:], w3t_s[:, ct, 0, k, :], rhs,
                                 start=(k == 0), stop=(k == 8))
            nc.scalar.activation(
                y3[:, ct, 2 * bp:2 * bp + 2, 1:17, 1:17],
                pst[:].rearrange("p (b h w) -> p b h w", b=2, h=16),
                AF.Relu, bias=cbs_s[:, ct:ct + 1])

    # --------------------------------------------------------- conv4

    w4p_cm, w4p = popen("w4p", 1)
    y4p_cm, y4p = popen("y4p", 1)
    pl3_cm, pl3 = popen("pl3", 2)
    w4t_s = w4p.tile([128, 2, 2, 9, 128], BF)
    for ct in range(2):
        nc.sync.dma_start(w4t_s[:, ct], d["w4t_d"][ct])
    y4r = y4p.tile([128, 2, 16, 18, 18], BF)
    nc.gpsimd.memset(y4r[:, :, :, 0:18:17, :], NEG)
    nc.gpsimd.memset(y4r[:, :, :, 1:17, 0:18:17], NEG)
    for ct in range(2):
        for bp in range(8):
            pst = psc.tile([128, 512], F32, tag="pc")
            first = True
            for kt in range(2):
                for k, (dh, dw) in enumerate(OFF9):
                    rhs = y3[:, kt, 2 * bp:2 * bp + 2, dh:dh + 16, dw:dw + 16]
                    nc.tensor.matmul(pst[:], w4t_s[:, ct, kt, k, :], rhs,
                                     start=first, stop=(kt == 1 and k == 8))
                    first = False
            nc.scalar.copy(
                y4r[:, ct, 2 * bp:2 * bp + 2, 1:17, 1:17],
                pst[:].rearrange("p (b h w) -> p b h w", b=2, h=16))

    # pool3 + bias+relu -> p3 [128, 2, 16, 10, 10]
    p3 = p3p.tile([128, 2, 16, 10, 10], BF)
    nc.gpsimd.memset(p3[:, :, :, 0:10:9, :], 0.0)
    nc.gpsimd.memset(p3[:, :, :, 1:9, 0:10:9], 0.0)
    for ct in range(2):
        t1 = pl3.tile([128, 16, 18, 8], BF, tag="t1c")
        nc.vector.tensor_tensor(t1[:], y4r[:, ct, :, :, 0:16:2],
                                y4r[:, ct, :, :, 1:17:2], ALU.max)
        nc.vector.tensor_tensor(t1[:], t1[:], y4r[:, ct, :, :, 2:18:2], ALU.max)
        t2 = pl3.tile([128, 16, 8, 8], BF, tag="t2c")
        nc.vector.tensor_tensor(t2[:], t1[:, :, 0:16:2, :], t1[:, :, 1:17:2, :], ALU.max)
        nc.vector.tensor_tensor(t2[:], t2[:], t1[:, :, 2:18:2, :], ALU.max)
        nc.scalar.activation(p3[:, ct, :, 1:9, 1:9], t2[:], AF.Relu,
                             bias=cbs_s[:, 2 + ct:3 + ct])

    pclose(pl3_cm, y4p_cm, w4p_cm, y3p_cm, w3p_cm)
    p4p_cm, p4p = popen("p4p", 1)
    y7p_cm, y7p_pool = popen("y7p", 1)
    cwA_cm, cwA = popen("cwA", 1)
    w1fA = cwA.tile([128, 8, 4, 5, 128], BF, tag="w1fA")
    cwB_cm, cwB = popen("cwB", 1)
    w1fB = cwB.tile([128, 8, 4, 5, 128], BF, tag="w1fB")
    y5p_cm, y5p = popen("y5p", 1)
    wstg_cm, wstg = popen("wstg", 2)
    pl4_cm, pl4 = popen("pl4", 2)

    # ----------------------------------------------------- conv5

    y5 = y5p.tile([128, 4, 16, 10, 10], BF, tag="y5")
    nc.gpsimd.memset(y5[:, :, :, 0:10:9, :], 0.0)
    nc.gpsimd.memset(y5[:, :, :, 1:9, 0:10:9], 0.0)
    for ct in range(4):
        w5c = wstg.tile([128, 2, 9, 128], BF, tag="wc5", name=f"w5c{ct}")
        nc.sync.dma_start(w5c[:], d["w5t_d"][ct])
        for bp in range(2):
            pst = psc.tile([128, 512], F32, tag="pc")
            first = True
            for kt in range(2):
                for k, (dh, dw) in enumerate(OFF9):
                    rhs = p3[:, kt, 8 * bp:8 * bp + 8, dh:dh + 8, dw:dw + 8]
                    nc.tensor.matmul(pst[:], w5c[:, kt, k, :], rhs,
                                     start=first, stop=(kt == 1 and k == 8))
                    first = False
            nc.scalar.activation(
                y5[:, ct, 8 * bp:8 * bp + 8, 1:9, 1:9],
                pst[:].rearrange("p (b h w) -> p b h w", b=8, h=8),
                AF.Relu, bias=cbs_s[:, 4 + ct:5 + ct])

    # ------------------------------------------------- conv6

    y6r = y5p.tile([128, 4, 16, 10, 10], BF, tag="y6r")
    nc.gpsimd.memset(y6r[:, :, :, 0:10:9, :], NEG)
    nc.gpsimd.memset(y6r[:, :, :, 1:9, 0:10:9], NEG)
    for ct in range(4):
        w6c = wstg.tile([128, 4, 9, 128], BF, tag="wc6", name=f"w6c{ct}")
        nc.sync.dma_start(w6c[:], d["w6t_d"][ct])
        nc.sync.dma_start(w1fA[:, 2 * ct:2 * ct + 2], d["w1f_d"][:, 2 * ct:2 * ct + 2])
        for bp in range(2):
            pst = psc.tile([128, 512], F32, tag="pc")
            first = True
            for kt in range(4):
                for k, (dh, dw) in enumerate(OFF9):
                    rhs = y5[:, kt, 8 * bp:8 * bp + 8, dh:dh + 8, dw:dw + 8]
                    nc.tensor.matmul(pst[:], w6c[:, kt, k, :], rhs,
                                     start=first, stop=(kt == 3 and k == 8))
                    first = False
            nc.scalar.copy(
                y6r[:, ct, 8 * bp:8 * bp + 8, 1:9, 1:9],
                pst[:].rearrange("p (b h w) -> p b h w", b=8, h=8))
    # pool4 + bias+relu -> p4 [128, 4, 16, 6, 6]
    p4 = p4p.tile([128, 4, 16, 6, 6], BF)
    nc.gpsimd.memset(p4[:, :, :, 0:6:5, :], 0.0)
    nc.gpsimd.memset(p4[:, :, :, 1:5, 0:6:5], 0.0)
    for ct in range(4):
        t1 = pl4.tile([128, 16, 10, 4], BF, tag="t1d")
        nc.vector.tensor_tensor(t1[:], y6r[:, ct, :, :, 0:8:2],
                                y6r[:, ct, :, :, 1:9:2], ALU.max)
        nc.vector.tensor_tensor(t1[:], t1[:], y6r[:, ct, :, :, 2:10:2], ALU.max)
        t2 = pl4.tile([128, 16, 4, 4], BF, tag="t2d")
        nc.vector.tensor_tensor(t2[:], t1[:, :, 0:8:2, :], t1[:, :, 1:9:2, :], ALU.max)
        nc.vector.tensor_tensor(t2[:], t2[:], t1[:, :, 2:10:2, :], ALU.max)
        nc.scalar.activation(p4[:, ct, :, 1:5, 1:5], t2[:], AF.Relu,
                             bias=cbs_s[:, 8 + ct:9 + ct])

    pclose(pl4_cm)


    # ------------------------------------------------- conv7
    y7 = y7p_pool.tile([128, 4, 16, 16], BF, name="y7")
    for ct in range(4):
        w7c = wstg.tile([128, 4, 9, 128], BF, tag="wc6", name=f"w7c{ct}")
        nc.sync.dma_start(w7c[:], d["w7t_d"][ct])
        nc.sync.dma_start(w1fB[:, 2 * ct:2 * ct + 2], d["w1f_d"][:, 8 + 2 * ct:10 + 2 * ct])
        pst = psc.tile([128, 256], F32, tag="pc7")
        first = True
        for kt in range(4):
            for k, (dh, dw) in enumerate(OFF9):
                rhs = p4[:, kt, :, dh:dh + 4, dw:dw + 4]
                nc.tensor.matmul(pst[:], w7c[:, kt, k, :], rhs,
                                 start=first, stop=(kt == 3 and k == 8))
                first = False
        nc.scalar.activation(
            y7[:, ct, :, :],
            pst[:].rearrange("p (b n) -> p b n", b=16),
            AF.Relu, bias=cbs_s[:, 12 + ct:13 + ct])

    pclose(psc_cm)
    pclose(wstg_cm, y5p_cm)

    # --------------------------------------------- colons

    colp_cm, colp = popen("colp", 1)
    hp_cm, hp = popen("hp", 3)
    sxp_cm, sxp = popen("sxp", 4)
    colw_cm, colw = popen("colw", 1)
    psU_cm, psU = popen("psU", 3, "PSUM")
    psL_cm, psL = popen("psL", 3, "PSUM")
    psT_cm, psT = popen("psT", 1, "PSUM")
    cwC_cm, cwC = popen("cwC", 1)
    wtA = cwC.tile([128, 8, 2, 5, 128], BF, tag="wtA")
    for cch in range(2):
        nc.sync.dma_start(wtA[:, 4 * cch:4 * cch + 4], d["wt_d"][:, 4 * cch:4 * cch + 4])
    wnp_cm, wnp = popen("wnp", 3)
    b1r_s = colw.tile([1, 16, 5, 128], BF)
    nc.sync.dma_start(b1r_s[:], d["b1r_d"][:])
    w2c_s = colw.tile([128, 16, 5, 10], BF)
    nc.sync.dma_start(w2c_s[:], d["w2c_d"][:])
    b2r_s = colw.tile([1, 16, 10], BF)
    nc.sync.dma_start(b2r_s[:], d["b2r_d"][:])
    ones1 = colw.tile([1, 16], BF)
    nc.gpsimd.memset(ones1[:], 1.0)

    U_sb = colp.tile([128, 16, 5, 16], F32)
    h1 = colp.tile([128, 16, 5, 16], BF)
    preds1 = colp.tile([16, 16, 10], F32)
    preds2 = colp.tile([16, 16, 10], F32)

    def colon_tail(n, h_t, logits_buf):
        L_ps = psL.tile([16, 10], F32, tag="lp")
        for kt in range(5):
            nc.tensor.matmul(L_ps[:], h_t[:, kt, :], w2c_s[:, n, kt, :],
                             start=(kt == 0), stop=False)
        nc.tensor.matmul(L_ps[:], ones1[:], b2r_s[:, n, :], start=False, stop=True)
        nc.scalar.copy(logits_buf[:, n, :], L_ps[:])

    def batched_softmax(logits_buf, preds_out):
        # softmax over the last dim for all 16 nodes in 6 ops
        mx = sxp.tile([16, 16], F32, tag="mx")
        nc.vector.tensor_reduce(mx[:], logits_buf[:], AX.X, ALU.max, negate=True)
        sh = sxp.tile([16, 16, 10], F32, tag="sh")
        nc.vector.tensor_tensor(sh[:], logits_buf[:],
                                mx[:, :, None].to_broadcast((16, 16, 10)), ALU.add)
        ex = sxp.tile([16, 16, 10], F32, tag="ex")
        nc.scalar.activation(ex[:], sh[:], AF.Exp)
        sm = sxp.tile([16, 16], F32, tag="sm")
        nc.vector.tensor_reduce(sm[:], ex[:], AX.X, ALU.add)
        rc = sxp.tile([16, 16], F32, tag="rc")
        nc.vector.reciprocal(rc[:], sm[:])
        nc.vector.tensor_tensor(preds_out[:], ex[:],
                                rc[:, :, None].to_broadcast((16, 16, 10)), ALU.mult)

    # pass 1
    logits1 = colp.tile([16, 16, 10], F32)
    logits2 = colp.tile([16, 16, 10], F32)
    for n in range(16):
        w1n = (w1fA if n < 8 else w1fB)[:, n % 8]
        up = psU.tile([128, 5, 16], F32, tag="up")
        for mt in range(5):
            for kt in range(4):
                nc.tensor.matmul(up[:, mt, :], w1n[:, kt, mt, :], y7[:, kt, :, n],
                                 start=(kt == 0), stop=False)
            nc.tensor.matmul(up[:, mt, :], b1r_s[:, n, mt, :], ones1[:],
                             start=False, stop=(mt == 4))
        nc.vector.tensor_copy(U_sb[:, n, :, :], up[:])
        nc.scalar.activation(h1[:, n, :, :], up[:], AF.Tanh)
        colon_tail(n, h1[:, n, :, :], logits1)
    batched_softmax(logits1, preds1)

    # transpose preds1 -> predsT [128|32, 16]
    pv = preds1[:].rearrange("p n c -> p (n c)")
    predsT = colp.tile([128, 2, 16], BF)
    nc.gpsimd.memset(predsT[:, 1, :], 0.0)
    ptp = psT.tile([128, 16], F32, tag="ptp")
    nc.tensor.transpose(ptp[:], pv[:, 0:128], ident[0:16, 0:16])
    nc.scalar.copy(predsT[:, 0, :], ptp[:])
    ptp2 = psT.tile([32, 16], F32, tag="ptp2")
    nc.tensor.transpose(ptp2[:], pv[:, 128:160], ident[0:16, 0:16])
    nc.scalar.copy(predsT[0:32, 1, :], ptp2[:])

    # pass 2
    for n in range(16):
        if n < 8:
            wtn = wtA[:, n]
        else:
            wtn = wnp.tile([128, 2, 5, 128], BF, tag="wtn", name=f"wtn{n}")
            nc.sync.dma_start(wtn[:], d["wt_d"][:, n])
        h2 = hp.tile([128, 5, 16], BF, tag="h2")
        dp = psU.tile([128, 5, 16], F32, tag="up")
        for mt in range(5):
            for kt in range(2):
                nc.tensor.matmul(dp[:, mt, :], wtn[:, kt, mt, :], predsT[:, kt, :],
                                 start=(kt == 0), stop=(kt == 1 and mt == 4))
        vv = sxp.tile([128, 5, 16], F32, tag="vv")
        nc.vector.tensor_tensor(vv[:], U_sb[:, n, :, :], dp[:], ALU.add)
        nc.scalar.activation(h2[:], vv[:], AF.Tanh)
        colon_tail(n, h2[:], logits2)
    batched_softmax(logits2, preds2)

    nc.sync.dma_start(d["out_d"][:], preds2[:])

    pclose(psT_cm, psL_cm, psU_cm)
    pclose(wnp_cm, cwC_cm, colw_cm, sxp_cm, hp_cm, colp_cm, cwB_cm, cwA_cm, y7p_cm,
           p4p_cm, p3p_cm, p2p_cm, p1p_cm, statp_cm, const_cm)


_NC_CACHE = None


def kernel(**inputs):
    global _NC_CACHE, _LAST_RESULT
    g = _prep(inputs)
    xi_full = g.pop("_xi_full")

    if _NC_CACHE is None:
        _NC_CACHE = _build()
    nc = _NC_CACHE

    in_maps = []
    for c in range(N_CORES):
        m = {"xi": np.ascontiguousarray(xi_full[:, c * BL:(c + 1) * BL])}
        m.update(g)
        in_maps.append(m)

    res = bass_utils.run_bass_kernel_spmd(nc, in_maps, core_ids=list(range(N_CORES)))
    _LAST_RESULT = res

    preds_all = np.concatenate([res.results[c]["preds2"] for c in range(N_CORES)], axis=0)
    preds2 = np.ascontiguousarray(preds_all.transpose(1, 0, 2)).astype(np.float32)
    sum_preds = preds2.mean(axis=0)
    return (sum_preds * sum_preds).astype(np.float32), preds2
